# revision 1
# baseline (speedup 1.0000x reference)
"""Trainium2 Bass kernel for nn_JanusModel (sparse_attention, GQA, two mask groups).

Sharding: core c in [0,8) handles batch b=c//4 and query-row block q0=(c%4)*512.
Each core computes all 16 heads for its 512 query rows -> disjoint output slices,
no collectives. All heavy operands are laid out on host (transposes/permutes only).

On-device math per core (ARCH-T, scores kept transposed [sk, sq]):
  qT/kT/v projections (fp32r matmuls), scores.T = K @ qT/8 (row-tiled head pairs),
  P = exp(scores) * exp(maskT) (ACT exp + DVE bf16 mul), AV col-tiled head pairs,
  rowsums via M=1 quad matmuls with a ones vector, divide, output projection.
"""

import os
import sys

import numpy as np

for _p in ("/opt/trn_rl_repo",):
    if os.path.isdir(_p) and _p not in sys.path:
        sys.path.insert(0, _p)

import concourse.bass as bass
import concourse.tile as tile
from concourse import bacc, mybir
from concourse.bass_utils import run_bass_kernel_spmd

B, S, D = 2, 2048, 1024
H, KVH, HD = 16, 4, 64
NCORES = 8
SQ = S // 4  # 512 query rows per core
P = 128
NKT = S // P  # 16 key tiles

# Head pairs: (a, b) share a kT tile; a uses kv head 2*(j//4), b uses +1.
PAIRS = [(0, 4), (1, 5), (2, 6), (3, 7), (8, 12), (9, 13), (10, 14), (11, 15)]

f32 = mybir.dt.float32
bf16 = mybir.dt.bfloat16
f32r = mybir.dt.float32r
EXP = mybir.ActivationFunctionType.Exp
DIV = mybir.AluOpType.divide

_CACHE = {}


def _r(ap):
    return ap.bitcast(f32r)


def _body(tc, xT, wqT, wkT, wvT, woT, mT, out):
    nc = tc.nc
    rs_dram = nc.dram_tensor("rs_scratch", [8, 2, SQ], f32).ap()
    xT_r = xT.rearrange("(c p) s -> c p s", p=P)        # [8,128,2048]
    wqT_r = wqT.rearrange("(c p) f -> c p f", p=P)      # [8,128,1024]
    wkT_r = wkT.rearrange("(c p) f -> c p f", p=P)      # [8,128,256]
    wvT_r = wvT.rearrange("(c p) f -> c p f", p=P)      # [8,128,256]
    woT_r = woT.rearrange("(c p) d -> c p d", p=P)      # [8,128,1024]
    mT_r = mT.rearrange("m (c p) q -> m p c q", p=P)    # [2,128,16,512]
    out_r = out.rearrange("(t p) d -> t p d", p=P)      # [4,128,1024]

    persist = tc.alloc_tile_pool(name="persist", bufs=1)
    qT_sb = persist.tile([P, 8, SQ], f32r, name="qT_sb")      # pair j: a rows 0:64, b rows 64:128
    kT_sb = persist.tile([P, 2, S], f32r, name="kT_sb")       # tile jt: kv 2jt rows 0:64, kv 2jt+1 rows 64:128
    v_sb = persist.tile([P, NKT, KVH * HD], bf16, name="v_sb")

    # ---------------- phase A: load x/w, projections ----------------
    with tc.tile_pool(name="xw", bufs=1) as xw, \
         tc.tile_pool(name="pps", bufs=4, space="PSUM") as pps:
        x_sb = xw.tile([P, 8, S], f32r, name="x_sb")
        wq_sb = xw.tile([P, 8, H * HD], f32r, name="wq_sb")
        wk_sb = xw.tile([P, 8, KVH * HD], f32r, name="wk_sb")
        wv_sb = xw.tile([P, 8, KVH * HD], f32r, name="wv_sb")
        for c in range(8):
            nc.gpsimd.dma_start(out=x_sb[:, c, :], in_=xT_r[c])
            nc.gpsimd.dma_start(out=wq_sb[:, c, :], in_=wqT_r[c])
            nc.gpsimd.dma_start(out=wk_sb[:, c, :], in_=wkT_r[c])
            nc.gpsimd.dma_start(out=wv_sb[:, c, :], in_=wvT_r[c])

        # q projection: out [128 qfeat(pair j), 512]; fold 1/sqrt(HD)=1/8 scale
        for j in range(8):
            ps = pps.tile([P, SQ], f32, tag="pq", name=f"psq{j}")
            for kc in range(8):
                nc.tensor.matmul(
                    ps, lhsT=_r(wq_sb[:, kc, j * P:(j + 1) * P]),
                    rhs=_r(x_sb[:, kc, 0:SQ]),
                    start=(kc == 0), stop=(kc == 7))
            nc.vector.tensor_scalar_mul(qT_sb[:, j, :], ps, 0.125)

        # k projection: kT tiles [128 kvfeat, 2048]
        for jt in range(2):
            for ns in range(4):
                ps = pps.tile([P, SQ], f32, tag="pq", name=f"psk{jt}{ns}")
                for kc in range(8):
                    nc.tensor.matmul(
                        ps, lhsT=_r(wk_sb[:, kc, jt * P:(jt + 1) * P]),
                        rhs=_r(x_sb[:, kc, ns * SQ:(ns + 1) * SQ]),
                        start=(kc == 0), stop=(kc == 7))
                nc.vector.tensor_copy(out=kT_sb[:, jt, ns * SQ:(ns + 1) * SQ], in_=ps)

        # v projection: natural [sk 128-tile, 256] -> bf16
        for t in range(NKT):
            ps = pps.tile([P, KVH * HD], f32, tag="pv", name=f"psv{t}")
            for kc in range(8):
                nc.tensor.matmul(
                    ps, lhsT=_r(x_sb[:, kc, t * P:(t + 1) * P]),
                    rhs=_r(wv_sb[:, kc, :]),
                    start=(kc == 0), stop=(kc == 7))
            nc.vector.tensor_copy(out=v_sb[:, t, :], in_=ps)

    # ---------------- phase B: masks exp, attention ----------------
    with tc.tile_pool(name="attn_sb", bufs=1) as asb:
        expm_sb = asb.tile([P, 2, NKT, SQ], bf16, name="expm_sb")
        attnT_sb = asb.tile([P, 8, SQ], f32r, name="attnT_sb")
        ones_bf = asb.tile([P, 1], bf16, name="ones_bf")
        nc.vector.memset(ones_bf, 1.0)

        with tc.tile_pool(name="ml", bufs=2) as mlp:
            for m in range(2):
                for tg in range(8):
                    ml = mlp.tile([P, 2, SQ], f32, tag="ml", name=f"ml{m}{tg}")
                    nc.sync.dma_start(out=ml, in_=mT_r[m, :, 2 * tg:2 * tg + 2, :])
                    nc.scalar.activation(
                        out=expm_sb[:, m, 2 * tg:2 * tg + 2, :], in_=ml, func=EXP)

        with tc.tile_pool(name="psA", bufs=1, space="PSUM") as psA, \
             tc.tile_pool(name="psB", bufs=1, space="PSUM") as psB, \
             tc.tile_pool(name="avp", bufs=1, space="PSUM") as avp, \
             tc.tile_pool(name="qdp", bufs=1, space="PSUM") as qdp, \
             tc.tile_pool(name="praw", bufs=3) as praw, \
             tc.tile_pool(name="ppool", bufs=4) as ppool, \
             tc.tile_pool(name="small", bufs=2) as small:
            quad = None
            for j, (ha, hb) in enumerate(PAIRS):
                jt = j // 4          # kT tile index
                m = j // 4           # mask index
                vca = (j // 4) * 2 * HD   # v column of kv head for a
                vcb = vca + HD
                if j % 2 == 0:
                    quad = qdp.tile([P, SQ], f32, tag="quad", name=f"quad{j}")
                ca = 64 * (j % 2)    # quad col for head a
                cb = ca + 32
                av = avp.tile([P, SQ], f32, tag="av", name=f"av{j}")
                pa_tiles, pb_tiles = [], []
                for g in range(6):
                    nt = min(3, NKT - 3 * g)
                    sA = psA.tile([P, 3, SQ], f32, tag="sA", name=f"sA{j}_{g}")
                    sB = psB.tile([P, 3, SQ], f32, tag="sB", name=f"sB{j}_{g}")
                    for i in range(nt):
                        t = 3 * g + i
                        nc.tensor.matmul(
                            sA[:, i, :], lhsT=_r(kT_sb[0:64, jt, t * P:(t + 1) * P]),
                            rhs=_r(qT_sb[0:64, j, :]), start=True, stop=True)
                        nc.tensor.matmul(
                            sB[:, i, :], lhsT=_r(kT_sb[64:128, jt, t * P:(t + 1) * P]),
                            rhs=_r(qT_sb[64:128, j, :]), start=True, stop=True)
                    prA = praw.tile([P, 3, SQ], bf16, tag="prA", name=f"prA{j}_{g}")
                    prB = praw.tile([P, 3, SQ], bf16, tag="prB", name=f"prB{j}_{g}")
                    nc.scalar.activation(out=prA[:, 0:nt, :], in_=sA[:, 0:nt, :], func=EXP)
                    nc.scalar.activation(out=prB[:, 0:nt, :], in_=sB[:, 0:nt, :], func=EXP)
                    pA = ppool.tile([P, 3, SQ], bf16, tag="pA", name=f"pA{j}_{g}")
                    pB = ppool.tile([P, 3, SQ], bf16, tag="pB", name=f"pB{j}_{g}")
                    nc.vector.tensor_mul(pA[:, 0:nt, :], prA[:, 0:nt, :],
                                         expm_sb[:, m, 3 * g:3 * g + nt, :])
                    nc.vector.tensor_mul(pB[:, 0:nt, :], prB[:, 0:nt, :],
                                         expm_sb[:, m, 3 * g:3 * g + nt, :])
                    pa_tiles.append(pA)
                    pb_tiles.append(pB)
                    # AV + rowsum consume this group's P tiles immediately
                    for i in range(nt):
                        t = 3 * g + i
                        st = (t == 0)
                        sp = (t == NKT - 1)
                        nc.tensor.matmul(av[0:64, :], lhsT=v_sb[:, t, vca:vca + HD],
                                         rhs=pA[:, i, :], start=st, stop=sp)
                        nc.tensor.matmul(av[64:128, :], lhsT=v_sb[:, t, vcb:vcb + HD],
                                         rhs=pB[:, i, :], start=st, stop=sp)
                        nc.tensor.matmul(quad[ca:ca + 1, :], lhsT=ones_bf[:, 0:1],
                                         rhs=pA[:, i, :], start=st, stop=sp,
                                         tile_position=(0, ca))
                        nc.tensor.matmul(quad[cb:cb + 1, :], lhsT=ones_bf[:, 0:1],
                                         rhs=pB[:, i, :], start=st, stop=sp,
                                         tile_position=(0, cb))
                # rowsums -> broadcast [128,512]; attnT = av / rs
                rs = small.tile([P, SQ], f32, tag="rs", name=f"rs{j}")
                nc.vector.tensor_copy(out=rs[ca:ca + 1, :], in_=quad[ca:ca + 1, :])
                nc.vector.tensor_copy(out=rs[cb:cb + 1, :], in_=quad[cb:cb + 1, :])
                nc.sync.dma_start(out=rs_dram[j, 0, :], in_=rs[ca:ca + 1, :])
                nc.sync.dma_start(out=rs_dram[j, 1, :], in_=rs[cb:cb + 1, :])
                bc = small.tile([P, SQ], f32, tag="bc", name=f"bc{j}")
                for half in range(2):
                    row = rs_dram[j, half, :]
                    bcast = bass.AP(tensor=row.tensor, offset=row.offset,
                                    ap=[[0, 64]] + list(row.ap))
                    nc.sync.dma_start(out=bc[64 * half:64 * half + 64, :], in_=bcast)
                nc.vector.reciprocal(out=bc, in_=bc)
                nc.vector.tensor_mul(attnT_sb[:, j, :], av, bc)

        # ---------------- phase C: output projection ----------------
        with tc.tile_pool(name="wo", bufs=2) as wop, \
             tc.tile_pool(name="ops", bufs=8, space="PSUM") as ops, \
             tc.tile_pool(name="osb", bufs=2) as osb:
            pso = [ops.tile([P, SQ], f32, tag="ops", name=f"pso{i}") for i in range(8)]
            for j in range(8):
                wo_sb = wop.tile([P, D], f32r, tag="wo", name=f"wo{j}")
                nc.gpsimd.dma_start(out=wo_sb, in_=woT_r[j])
                for st in range(4):
                    for nt in range(2):
                        nc.tensor.matmul(
                            pso[st * 2 + nt],
                            lhsT=_r(attnT_sb[:, j, st * P:(st + 1) * P]),
                            rhs=_r(wo_sb[:, nt * SQ:(nt + 1) * SQ]),
                            start=(j == 0), stop=(j == 7))
            for st in range(4):
                ob = osb.tile([P, D], f32, tag="ob", name=f"ob{st}")
                nc.vector.tensor_copy(out=ob[:, 0:SQ], in_=pso[st * 2])
                nc.vector.tensor_copy(out=ob[:, SQ:D], in_=pso[st * 2 + 1])
                nc.sync.dma_start(out=out_r[st], in_=ob)
    persist.release()


def _build():
    if "nc" in _CACHE:
        return _CACHE["nc"]
    nc = bacc.Bacc("TRN2", target_bir_lowering=False, debug=False)
    xT = nc.dram_tensor("xT", [D, S], f32, kind="ExternalInput").ap()
    wqT = nc.dram_tensor("wqT", [D, H * HD], f32, kind="ExternalInput").ap()
    wkT = nc.dram_tensor("wkT", [D, KVH * HD], f32, kind="ExternalInput").ap()
    wvT = nc.dram_tensor("wvT", [D, KVH * HD], f32, kind="ExternalInput").ap()
    woT = nc.dram_tensor("woT", [H * HD, D], f32, kind="ExternalInput").ap()
    mT = nc.dram_tensor("mT", [2, S, SQ], f32, kind="ExternalInput").ap()
    out = nc.dram_tensor("out", [SQ, D], f32, kind="ExternalOutput").ap()
    with tile.TileContext(nc) as tc:
        _body(tc, xT, wqT, wkT, wvT, woT, mT, out)
    nc.compile()
    _CACHE["nc"] = nc
    return nc


def _host_prep(hidden_states, full_mask, tag_mask, wq, wk, wv, wo):
    # pair-ordered feature permutation for wq columns / wo.T rows
    perm = np.concatenate([np.r_[a * HD:(a + 1) * HD, b * HD:(b + 1) * HD]
                           for a, b in PAIRS])
    wqT = np.ascontiguousarray(wq.T[:, perm], np.float32)      # [D, 1024]
    wkT = np.ascontiguousarray(wk.T, np.float32)               # [D, 256]
    wvT = np.ascontiguousarray(wv.T, np.float32)               # [D, 256]
    woT = np.ascontiguousarray(wo.T[perm, :], np.float32)      # [1024, D]
    masksT = [np.ascontiguousarray(full_mask[b, 0].T) for b in range(B)] + \
             [np.ascontiguousarray(tag_mask[b, 0].T) for b in range(B)]
    xTs = [np.ascontiguousarray(hidden_states[b].T, np.float32) for b in range(B)]
    in_maps = []
    for c in range(NCORES):
        b, q0 = c // 4, (c % 4) * SQ
        xT_c = np.roll(xTs[b], -q0, axis=1)
        fmT = np.roll(masksT[b][:, q0:q0 + SQ], -q0, axis=0)
        tgT = np.roll(masksT[2 + b][:, q0:q0 + SQ], -q0, axis=0)
        mT_c = np.ascontiguousarray(np.stack([fmT, tgT]), np.float32)
        in_maps.append({"xT": np.ascontiguousarray(xT_c), "wqT": wqT, "wkT": wkT,
                        "wvT": wvT, "woT": woT, "mT": mT_c})
    return in_maps


def kernel(hidden_states, full_mask, tag_mask, wq, wk, wv, wo, _trace=False):
    args = [np.asarray(a, np.float32) for a in
            (hidden_states, full_mask, tag_mask, wq, wk, wv, wo)]
    nc = _build()
    in_maps = _host_prep(*args)
    try:
        res = run_bass_kernel_spmd(nc, in_maps, core_ids=list(range(NCORES)),
                                   trace=_trace)
    except ModuleNotFoundError:
        res = run_bass_kernel_spmd(nc, in_maps, core_ids=list(range(NCORES)))
    _CACHE["last_results"] = res
    full = np.empty((B, S, D), np.float32)
    for c in range(NCORES):
        b, q0 = c // 4, (c % 4) * SQ
        full[b, q0:q0 + SQ, :] = res.results[c]["out"]
    return full



# revision 3
# speedup vs baseline: 31233.4679x; 31233.4679x over previous
"""Trainium2 Bass kernel for nn_JanusModel (sparse_attention, GQA, two mask groups).

Sharding: core c in [0,8) handles batch b=c//4 and query-row block q0=(c%4)*512.
Each core computes all 16 heads for its 512 query rows -> disjoint output slices,
no collectives. Heavy operands laid out on host (transposes/permutes, exp(mask),
1/sqrt(hd) folded into wq, bf16 casts).

On-device math per core (all bf16 operands, fp32 PSUM):
  qT/kT/v projections, scoresT = K @ qT (row-tiled head pairs, [sk, sq] layout),
  P = exp(scores) * expm (ACT exp + DVE bf16 mul), AV matmuls with a ones-column
  appended to V so row 64 of each AV psum accumulates the softmax denominator,
  reciprocal + DMA broadcast, output projection accumulated over head pairs.
"""

import os
import sys

import numpy as np

for _p in ("/opt/trn_rl_repo",):
    if os.path.isdir(_p) and _p not in sys.path:
        sys.path.insert(0, _p)

import concourse.bass as bass
import concourse.tile as tile
from concourse import bacc, mybir
from concourse.bass_utils import run_bass_kernel_spmd

B, S, D = 2, 2048, 1024
H, KVH, HD = 16, 4, 64
NCORES = 8
SQ = S // 4          # 512 query rows per core
P = 128
NKT = S // P         # 16 key tiles
KC = D // P          # 8 contraction chunks for projections

# Head pairs: (a, b) share a kT tile; a uses kv head 2*(j//4), b uses +1.
PAIRS = [(0, 4), (1, 5), (2, 6), (3, 7), (8, 12), (9, 13), (10, 14), (11, 15)]
GROUPS = [(0, 3), (3, 3), (6, 3), (9, 3), (12, 3), (15, 1)]  # (t0, nt) k-tile groups

f32 = mybir.dt.float32
bf16 = mybir.dt.bfloat16
EXP = mybir.ActivationFunctionType.Exp

_CACHE = {}


def _body(tc, xT, wqT, wkT, wvT, woT, emT, out):
    nc = tc.nc
    rs_dram = nc.dram_tensor("rs_scratch", [8, 2, SQ], f32).ap()
    xT_r = xT.rearrange("(kc p) (c s) -> c p kc s", p=P, s=SQ)   # [4,128,8,512]
    wqT_r = wqT.rearrange("(kc p) f -> p kc f", p=P)             # [128,8,1024]
    wkT_r = wkT.rearrange("(kc p) f -> p kc f", p=P)             # [128,8,256]
    wvT_r = wvT.rearrange("(kc p) f -> p kc f", p=P)             # [128,8,256]
    woT_r = woT.rearrange("(j p) d -> p j d", p=P)               # [128,8,1024]
    emT_r = emT.rearrange("m (t p) q -> m p t q", p=P)           # [2,128,16,512]
    out_r = out.rearrange("(t p) (n q) -> t n p q", p=P, q=SQ)   # [4,2,128,512]

    persist = tc.alloc_tile_pool(name="persist", bufs=1)
    qT_sb = persist.tile([P, 8, SQ], bf16, name="qT_sb")      # pair j: a 0:64, b 64:128
    kT_sb = persist.tile([P, 2, S], bf16, name="kT_sb")       # jt: kv 2jt 0:64, 2jt+1 64:128
    v_sb = persist.tile([P, NKT, KVH, HD + 1], bf16, name="v_sb")  # col HD = ones
    em_sb = persist.tile([P, 2, NKT, SQ], bf16, name="em_sb")
    attnT_sb = persist.tile([P, 8, SQ], bf16, name="attnT_sb")
    wo_sb = persist.tile([P, 8, D], bf16, name="wo_sb")
    warm = persist.tile([1, 2], bf16, name="warm")

    # ---- all input DMAs, ordered for earliest compute start (one SP queue) ----
    with tc.tile_pool(name="xw", bufs=1) as xw:
        x_sb = xw.tile([P, KC, S], bf16, name="x_sb")
        wq_sb = xw.tile([P, KC, H * HD], bf16, name="wq_sb")
        wk_sb = xw.tile([P, KC, KVH * HD], bf16, name="wk_sb")
        wv_sb = xw.tile([P, KC, KVH * HD], bf16, name="wv_sb")
        nc.sync.dma_start(out=x_sb[:, :, 0:SQ], in_=xT_r[0])
        nc.sync.dma_start(out=wk_sb, in_=wkT_r)
        nc.sync.dma_start(out=wq_sb, in_=wqT_r)
        nc.sync.dma_start(out=wv_sb, in_=wvT_r)
        for c in range(1, 4):
            nc.sync.dma_start(out=x_sb[:, :, c * SQ:(c + 1) * SQ], in_=xT_r[c])
        nc.sync.dma_start(out=em_sb[:, 0], in_=emT_r[0])
        nc.sync.dma_start(out=em_sb[:, 1], in_=emT_r[1])
        nc.sync.dma_start(out=wo_sb, in_=woT_r)

        # ACT exp table warm-up + ones column of v_sb
        nc.vector.memset(warm, 0.0)
        nc.scalar.activation(out=warm, in_=warm, func=EXP)
        nc.vector.memset(v_sb[:, :, :, HD:HD + 1], 1.0)

        # ---------------- phase A: projections ----------------
        with tc.tile_pool(name="pps", bufs=2, space="PSUM") as pps:
            for s in range(8):
                jt, ns = s // 4, s % 4
                # k projection chunk: kT[:, jt, ns*SQ:(ns+1)*SQ]
                ps = pps.tile([P, SQ], f32, tag="pk", name=f"psk{s}")
                for kc in range(KC):
                    nc.tensor.matmul(
                        ps, lhsT=wk_sb[:, kc, jt * P:(jt + 1) * P],
                        rhs=x_sb[:, kc, ns * SQ:(ns + 1) * SQ],
                        start=(kc == 0), stop=(kc == KC - 1))
                nc.scalar.copy(out=kT_sb[:, jt, ns * SQ:(ns + 1) * SQ], in_=ps)
                # q projection for pair s (scale folded into wq on host)
                ps = pps.tile([P, SQ], f32, tag="pk", name=f"psq{s}")
                for kc in range(KC):
                    nc.tensor.matmul(
                        ps, lhsT=wq_sb[:, kc, s * P:(s + 1) * P],
                        rhs=x_sb[:, kc, 0:SQ],
                        start=(kc == 0), stop=(kc == KC - 1))
                nc.scalar.copy(out=qT_sb[:, s, :], in_=ps)
                # v projection tiles 2s, 2s+1 (natural [sk, feat] layout)
                for t in (2 * s, 2 * s + 1):
                    ps = pps.tile([P, KVH * HD], f32, tag="pv", name=f"psv{t}")
                    for kc in range(KC):
                        nc.tensor.matmul(
                            ps, lhsT=x_sb[:, kc, t * P:(t + 1) * P],
                            rhs=wv_sb[:, kc, :],
                            start=(kc == 0), stop=(kc == KC - 1))
                    nc.vector.tensor_copy(
                        out=v_sb[:, t, :, 0:HD],
                        in_=ps.rearrange("p (k h) -> p k h", h=HD))

        # ---------------- phase B: attention ----------------
        with tc.tile_pool(name="psc", bufs=2, space="PSUM") as psc, \
             tc.tile_pool(name="pav", bufs=1, space="PSUM") as pav, \
             tc.tile_pool(name="prp", bufs=3) as prp, \
             tc.tile_pool(name="ppp", bufs=3) as ppp, \
             tc.tile_pool(name="small", bufs=2) as small:
            for j in range(8):
                jt = j // 4
                m = j // 4
                av_a = pav.tile([P, SQ], f32, tag="ava", name=f"ava{j}")
                av_b = pav.tile([P, SQ], f32, tag="avb", name=f"avb{j}")
                for (t0, ntg) in GROUPS:
                    for half in range(2):
                        kv = 2 * jt + half
                        av = av_a if half == 0 else av_b
                        r0, r1 = 64 * half, 64 * half + 64
                        sc = psc.tile([P, 3, SQ], f32, tag="sc", name=f"sc{j}_{t0}_{half}")
                        for i in range(ntg):
                            t = t0 + i
                            nc.tensor.matmul(
                                sc[:, i, :],
                                lhsT=kT_sb[r0:r1, jt, t * P:(t + 1) * P],
                                rhs=qT_sb[r0:r1, j, :], start=True, stop=True)
                        pr = prp.tile([P, 3, SQ], bf16, tag="pr", name=f"pr{j}_{t0}_{half}")
                        nc.scalar.activation(
                            out=pr[:, 0:ntg, :], in_=sc[:, 0:ntg, :], func=EXP)
                        pp = ppp.tile([P, 3, SQ], bf16, tag="pp", name=f"pp{j}_{t0}_{half}")
                        nc.vector.tensor_mul(
                            pp[:, 0:ntg, :], pr[:, 0:ntg, :],
                            em_sb[:, m, t0:t0 + ntg, :])
                        for i in range(ntg):
                            t = t0 + i
                            nc.tensor.matmul(
                                av[0:HD + 1, :], lhsT=v_sb[:, t, kv, :],
                                rhs=pp[:, i, :],
                                start=(t == 0), stop=(t == NKT - 1))
                # rowsums live in row HD of each av psum; reciprocal + broadcast
                rr = small.tile([P, 2, SQ], f32, tag="rr", name=f"rr{j}")
                nc.vector.reciprocal(out=rr[HD:HD + 1, 0, :], in_=av_a[HD:HD + 1, :])
                nc.vector.reciprocal(out=rr[HD:HD + 1, 1, :], in_=av_b[HD:HD + 1, :])
                nc.sync.dma_start(out=rs_dram[j], in_=rr[HD:HD + 1, :, :])
                bc = small.tile([P, SQ], f32, tag="bc", name=f"bc{j}")
                for half in range(2):
                    row = rs_dram[j, half, :]
                    bcast = bass.AP(tensor=row.tensor, offset=row.offset,
                                    ap=[[0, 64]] + list(row.ap))
                    nc.sync.dma_start(out=bc[64 * half:64 * half + 64, :], in_=bcast)
                nc.vector.tensor_mul(attnT_sb[0:64, j, :], av_a[0:HD, :], bc[0:64, :])
                nc.vector.tensor_mul(attnT_sb[64:128, j, :], av_b[0:HD, :], bc[64:128, :])

        # ---------------- phase C: output projection ----------------
        with tc.tile_pool(name="pop", bufs=2, space="PSUM") as pop, \
             tc.tile_pool(name="osb", bufs=2) as osb:
            for ch in range(8):
                st, nt2 = ch // 2, ch % 2
                po = pop.tile([P, SQ], f32, tag="po", name=f"po{ch}")
                for j in range(8):
                    nc.tensor.matmul(
                        po, lhsT=attnT_sb[:, j, st * P:(st + 1) * P],
                        rhs=wo_sb[:, j, nt2 * SQ:(nt2 + 1) * SQ],
                        start=(j == 0), stop=(j == 7))
                ob = osb.tile([P, SQ], f32, tag="ob", name=f"ob{ch}")
                nc.vector.tensor_copy(out=ob, in_=po)
                nc.sync.dma_start(out=out_r[st, nt2], in_=ob)
    persist.release()


def _build():
    if "nc" in _CACHE:
        return _CACHE["nc"]
    nc = bacc.Bacc("TRN2", target_bir_lowering=False, debug=False)
    xT = nc.dram_tensor("xT", [D, S], bf16, kind="ExternalInput").ap()
    wqT = nc.dram_tensor("wqT", [D, H * HD], bf16, kind="ExternalInput").ap()
    wkT = nc.dram_tensor("wkT", [D, KVH * HD], bf16, kind="ExternalInput").ap()
    wvT = nc.dram_tensor("wvT", [D, KVH * HD], bf16, kind="ExternalInput").ap()
    woT = nc.dram_tensor("woT", [H * HD, D], bf16, kind="ExternalInput").ap()
    emT = nc.dram_tensor("emT", [2, S, SQ], bf16, kind="ExternalInput").ap()
    out = nc.dram_tensor("out", [SQ, D], f32, kind="ExternalOutput").ap()
    with tile.TileContext(nc) as tc:
        _body(tc, xT, wqT, wkT, wvT, woT, emT, out)
    nc.compile()
    _CACHE["nc"] = nc
    return nc


def _host_prep(hidden_states, full_mask, tag_mask, wq, wk, wv, wo):
    # pair-ordered feature permutation for wq columns / wo.T rows
    perm = np.concatenate([np.r_[a * HD:(a + 1) * HD, b * HD:(b + 1) * HD]
                           for a, b in PAIRS])
    import ml_dtypes
    bf = ml_dtypes.bfloat16
    wqT = np.ascontiguousarray(wq.T[:, perm] * 0.125).astype(bf)   # [D, 1024]
    wkT = np.ascontiguousarray(wk.T).astype(bf)                    # [D, 256]
    wvT = np.ascontiguousarray(wv.T).astype(bf)                    # [D, 256]
    woT = np.ascontiguousarray(wo.T[perm, :]).astype(bf)           # [1024, D]
    # exp(mask) transposed to [sk, sq], rolled per core
    emasks = [np.exp(full_mask[b, 0].T) for b in range(B)] + \
             [np.exp(tag_mask[b, 0].T) for b in range(B)]
    xTs = [np.ascontiguousarray(hidden_states[b].T) for b in range(B)]
    in_maps = []
    for c in range(NCORES):
        b, q0 = c // 4, (c % 4) * SQ
        xT_c = np.roll(xTs[b], -q0, axis=1).astype(bf)
        fmT = np.roll(emasks[b][:, q0:q0 + SQ], -q0, axis=0)
        tgT = np.roll(emasks[2 + b][:, q0:q0 + SQ], -q0, axis=0)
        emT_c = np.ascontiguousarray(np.stack([fmT, tgT])).astype(bf)
        in_maps.append({"xT": np.ascontiguousarray(xT_c), "wqT": wqT, "wkT": wkT,
                        "wvT": wvT, "woT": woT, "emT": emT_c})
    return in_maps


def kernel(hidden_states, full_mask, tag_mask, wq, wk, wv, wo, _trace=False):
    args = [np.asarray(a, np.float32) for a in
            (hidden_states, full_mask, tag_mask, wq, wk, wv, wo)]
    nc = _build()
    in_maps = _host_prep(*args)
    try:
        res = run_bass_kernel_spmd(nc, in_maps, core_ids=list(range(NCORES)),
                                   trace=_trace)
    except ModuleNotFoundError:
        res = run_bass_kernel_spmd(nc, in_maps, core_ids=list(range(NCORES)))
    _CACHE["last_results"] = res
    full = np.empty((B, S, D), np.float32)
    for c in range(NCORES):
        b, q0 = c // 4, (c % 4) * SQ
        full[b, q0:q0 + SQ, :] = res.results[c]["out"]
    return full


# revision 22
# speedup vs baseline: 32644.0538x; 1.0452x over previous
"""Trainium2 Bass kernel for nn_JanusModel (sparse_attention, GQA, two mask groups).

Sharding: core c in [0,8) handles batch b=c//4 and query-row block q0=(c%4)*512.
Each core computes all 16 heads for its 512 query rows -> disjoint output slices,
no collectives. Heavy operands laid out on host (transposes/permutes, exp(mask),
1/sqrt(hd) folded into wq, bf16 casts).

On-device math per core (all bf16 operands, fp32 PSUM):
  qT/kT/v projections, scoresT = K @ qT (row-tiled head pairs, [sk, sq] layout),
  P = exp(scores) * expm (ACT exp + DVE bf16 mul), AV matmuls with a ones-column
  appended to V so row 64 of each AV psum accumulates the softmax denominator,
  reciprocal + DMA broadcast, output projection accumulated over head pairs.

Pipelining: pair 0's whole attention is interleaved into the projection loop so
the ACT engine starts exp work ~10us in; pairs 1-7 run a software-pipelined
PE emission (scores of group g+1 issued before AV of group g); attnT division
for pair j is deferred to pair j+2 so the rowsum DMA roundtrip never stalls PE.
"""

import os
import sys

import numpy as np

for _p in ("/opt/trn_rl_repo",):
    if os.path.isdir(_p) and _p not in sys.path:
        sys.path.insert(0, _p)

import concourse.bass as bass
import concourse.tile as tile
from concourse import bacc, mybir
from concourse.bass_utils import run_bass_kernel_spmd

B, S, D = 2, 2048, 1024
H, KVH, HD = 16, 4, 64
NCORES = 8
SQ = S // 4          # 512 query rows per core
P = 128
NKT = S // P         # 16 key tiles
KC = D // P          # 8 contraction chunks for projections
NG = 8               # 2-tile k groups per half

# Head pairs: (a, b) share a kT tile; a uses kv head 2*(j//4), b uses +1.
PAIRS = [(0, 4), (1, 5), (2, 6), (3, 7), (8, 12), (9, 13), (10, 14), (11, 15)]

f32 = mybir.dt.float32
bf16 = mybir.dt.bfloat16
EXP = mybir.ActivationFunctionType.Exp

_CACHE = {}


def _body(tc, xT, wqT, wkT, wvT, woT, emT, out):
    nc = tc.nc
    rs_dram = nc.dram_tensor("rs_scratch", [8, 2, SQ], f32).ap()
    xT_r = xT.rearrange("(kc p) (c s) -> c p kc s", p=P, s=SQ)   # [4,128,8,512]
    wqT_r = wqT.rearrange("j (p kc) f -> j p kc f", p=P)         # [8,128,8,128]
    wkT_r = wkT.rearrange("(kc p) f -> p kc f", p=P)             # [128,8,256]
    wvT_r = wvT.rearrange("(kc p) f -> p kc f", p=P)             # [128,8,256]
    woT_r = woT.rearrange("(j p) d -> p j d", p=P)               # [128,8,1024]
    emT_r = emT.rearrange("m (t p) q -> m p t q", p=P)           # [2,128,16,512]
    out_r = out.rearrange("(t p) (n q) -> t n p q", p=P, q=SQ)   # [4,2,128,512]

    persist = tc.alloc_tile_pool(name="persist", bufs=1)
    qT_sb = persist.tile([P, 8, SQ], bf16, name="qT_sb")      # pair j: a 0:64, b 64:128
    kT_sb = persist.tile([P, 2, S], bf16, name="kT_sb")       # jt: kv 2jt 0:64, 2jt+1 64:128
    v_sb = persist.tile([P, NKT, KVH, HD + 1], bf16, name="v_sb")  # col HD = ones
    em_sb = persist.tile([P, 2, NKT, SQ], bf16, name="em_sb")
    attnT_sb = persist.tile([P, 8, SQ], bf16, name="attnT_sb")
    wo_sb = persist.tile([P, 8, D], bf16, name="wo_sb")
    warm = persist.tile([1, 2], bf16, name="warm")

    # ---- all input DMAs, ordered for earliest compute start (one SP queue) ----
    pav = tc.alloc_tile_pool(name="pav", bufs=2, space="PSUM")
    with tc.tile_pool(name="xw", bufs=1) as xw, \
         tc.tile_pool(name="prp", bufs=3) as prp, \
         tc.tile_pool(name="ppp", bufs=3) as ppp, \
         tc.tile_pool(name="small", bufs=2) as small:
        x_sb = xw.tile([P, KC, S], bf16, name="x_sb")
        wq_sb = xw.tile([P, 8, KC, P], bf16, name="wq_sb")
        wk_sb = xw.tile([P, KC, KVH * HD], bf16, name="wk_sb")
        wv_sb = xw.tile([P, KC, KVH * HD], bf16, name="wv_sb")
        nc.sync.dma_start(out=wv_sb, in_=wvT_r)
        nc.sync.dma_start(out=x_sb[:, :, 0:P], in_=xT_r[0, :, :, 0:P])
        nc.sync.dma_start(out=x_sb[:, :, P:SQ], in_=xT_r[0, :, :, P:SQ])
        nc.sync.dma_start(out=wk_sb, in_=wkT_r)
        nc.sync.dma_start(out=wq_sb[:, 0], in_=wqT_r[0])
        nc.sync.dma_start(out=em_sb[:, 0, 0:8, :], in_=emT_r[0, :, 0:8, :])
        nc.sync.dma_start(out=x_sb[:, :, SQ:2 * SQ], in_=xT_r[1])
        nc.sync.dma_start(out=wq_sb[:, 1], in_=wqT_r[1])
        nc.sync.dma_start(out=em_sb[:, 0, 8:16, :], in_=emT_r[0, :, 8:16, :])
        nc.sync.dma_start(out=x_sb[:, :, 2 * SQ:3 * SQ], in_=xT_r[2])
        for j in (2, 3):
            nc.sync.dma_start(out=wq_sb[:, j], in_=wqT_r[j])
        nc.sync.dma_start(out=x_sb[:, :, 3 * SQ:4 * SQ], in_=xT_r[3])
        for j in (4, 5, 6, 7):
            nc.sync.dma_start(out=wq_sb[:, j], in_=wqT_r[j])
        nc.sync.dma_start(out=em_sb[:, 1], in_=emT_r[1])
        nc.sync.dma_start(out=wo_sb, in_=woT_r)

        # ACT exp table warm-up + ones column of v_sb
        nc.vector.memset(warm, 0.0)
        nc.scalar.activation(out=warm, in_=warm, func=EXP)
        nc.vector.memset(v_sb[:, :, :, HD:HD + 1], 1.0)

        # ---------- emission helpers ----------
        def emit_scores(pool, tag, j, g, half):
            """2 score matmuls for k-tiles 2g,2g+1 -> exp -> mask mul; returns pp."""
            jt = j // 4
            r0, r1 = 64 * half, 64 * half + 64
            sc = pool.tile([P, 2, SQ], f32, tag=tag, name=f"sc{j}_{g}_{half}")
            for i in range(2):
                t = 2 * g + i
                nc.tensor.matmul(
                    sc[:, i, :], lhsT=kT_sb[r0:r1, jt, t * P:(t + 1) * P],
                    rhs=qT_sb[r0:r1, j, :], start=True, stop=True)
            pr = prp.tile([P, 2, SQ], bf16, tag="pr", name=f"pr{j}_{g}_{half}")
            nc.scalar.activation(out=pr, in_=sc, func=EXP)
            pp = ppp.tile([P, 2, SQ], bf16, tag="pp", name=f"pp{j}_{g}_{half}")
            nc.vector.tensor_mul(pp, pr, em_sb[:, j // 4, 2 * g:2 * g + 2, :])
            return pp

        def emit_av(av, j, g, half, pp):
            kv = 2 * (j // 4) + half
            for i in range(2):
                t = 2 * g + i
                nc.tensor.matmul(
                    av[0:HD + 1, :], lhsT=v_sb[:, t, kv, :], rhs=pp[:, i, :],
                    start=(t == 0), stop=(t == NKT - 1))

        def emit_rowsum(j, av_a, av_b):
            """reciprocal of row HD, DMA roundtrip broadcast; returns (bc, av_a, av_b).

            The roundtrip latency is hidden by deferring the attnT division
            to pair j+2 (av pool is double-buffered)."""
            rr = small.tile([P, 2, SQ], f32, tag="rr", name=f"rr{j}")
            nc.vector.reciprocal(out=rr[HD:HD + 1, 0, :], in_=av_a[HD:HD + 1, :])
            nc.vector.reciprocal(out=rr[HD:HD + 1, 1, :], in_=av_b[HD:HD + 1, :])
            nc.sync.dma_start(out=rs_dram[j], in_=rr[HD:HD + 1, :, :])
            bc = small.tile([P, SQ], f32, tag="bc", name=f"bc{j}")
            for half in range(2):
                row = rs_dram[j, half, :]
                bcast = bass.AP(tensor=row.tensor, offset=row.offset,
                                ap=[[0, 64]] + list(row.ap))
                nc.sync.dma_start(out=bc[64 * half:64 * half + 64, :], in_=bcast)
            return bc, av_a, av_b

        def emit_attnT(j, pend):
            bc, av_a, av_b = pend
            nc.vector.tensor_mul(attnT_sb[0:64, j, :], av_a[0:HD, :], bc[0:64, :])
            nc.vector.tensor_mul(attnT_sb[64:128, j, :], av_b[0:HD, :], bc[64:128, :])

        # ---------------- phase A + pairs 0,1 interleaved ----------------
        # Pairs 0 and 1 share one score psum buffer (exps ping-pong through
        # it), keeping ACT busy while the PE runs projection chains. AV
        # consumption lags scores until the v tiles exist; leftover pp tiles
        # are retained in the ppp pool.
        pend = {}
        pp_store = {}
        sc_next = {0: 0, 1: 0}
        av_next = {0: 0, 1: 0}
        with tc.tile_pool(name="pps", bufs=2, space="PSUM") as pps, \
             tc.tile_pool(name="psc0", bufs=1, space="PSUM") as psc0:
            avs = {j: (pav.tile([P, SQ], f32, tag="ava", name=f"ava{j}"),
                       pav.tile([P, SQ], f32, tag="avb", name=f"avb{j}"))
                   for j in (0, 1)}

            def pump(s, budget):
                # consume: AV for groups whose v tiles exist (g <= s-1)
                for j in (0, 1):
                    while av_next[j] < sc_next[j] and av_next[j] <= s - 1:
                        g = av_next[j]
                        for half in (0, 1):
                            pp = pp_store.pop((j, g, half))
                            emit_av(avs[j][half], j, g, half, pp)
                        av_next[j] += 1
                # produce: scores for groups whose kT chunks exist (g <= 2s-1)
                n = 0
                while n < budget:
                    cands = [j for j in (0, 1)
                             if sc_next[j] < NG and sc_next[j] <= 2 * s - 1
                             and s >= max(1, j)]
                    if not cands:
                        break
                    j = min(cands, key=lambda jj: sc_next[jj])
                    g = sc_next[j]
                    for half in (0, 1):
                        pp_store[(j, g, half)] = emit_scores(psc0, "sc", j, g, half)
                    sc_next[j] += 1
                    n += 1

            BUDGET = [0, 3, 3, 2, 2, 2, 2, 2]
            for s in range(8):
                jt, ns = s // 4, s % 4
                # v projection tiles 2s, 2s+1 (natural [sk, feat] layout)
                for t in (2 * s, 2 * s + 1):
                    ps = pps.tile([P, KVH * HD], f32, tag="pa", name=f"psv{t}")
                    for kc in range(KC):
                        nc.tensor.matmul(
                            ps, lhsT=x_sb[:, kc, t * P:(t + 1) * P],
                            rhs=wv_sb[:, kc, :],
                            start=(kc == 0), stop=(kc == KC - 1))
                    nc.vector.tensor_copy(
                        out=v_sb[:, t, :, 0:HD],
                        in_=ps.rearrange("p (k h) -> p k h", h=HD))
                # k projection chunk: kT[:, jt, ns*SQ:(ns+1)*SQ]
                ps = pps.tile([P, SQ], f32, tag="pa", name=f"psk{s}")
                for kc in range(KC):
                    nc.tensor.matmul(
                        ps, lhsT=wk_sb[:, kc, jt * P:(jt + 1) * P],
                        rhs=x_sb[:, kc, ns * SQ:(ns + 1) * SQ],
                        start=(kc == 0), stop=(kc == KC - 1))
                nc.vector.tensor_copy(out=kT_sb[:, jt, ns * SQ:(ns + 1) * SQ], in_=ps)
                # q projection for pair s (scale folded into wq on host)
                ps = pps.tile([P, SQ], f32, tag="pa", name=f"psq{s}")
                for kc in range(KC):
                    nc.tensor.matmul(
                        ps, lhsT=wq_sb[:, s, kc, :], rhs=x_sb[:, kc, 0:SQ],
                        start=(kc == 0), stop=(kc == KC - 1))
                nc.vector.tensor_copy(out=qT_sb[:, s, :], in_=ps)
                pump(s, BUDGET[s])
            pump(8, 0)  # drain remaining AV work for pairs 0,1
            assert not pp_store and av_next == {0: NG, 1: NG}
            pend[0] = emit_rowsum(0, *avs[0])
            pend[1] = emit_rowsum(1, *avs[1])

        # ---------------- phase B: pairs 2-7, software pipelined ----------------
        with tc.tile_pool(name="psc1", bufs=1, space="PSUM") as psc1:
            for j in range(2, 8):
                if j - 2 in pend:
                    emit_attnT(j - 2, pend.pop(j - 2))
                av_a = pav.tile([P, SQ], f32, tag="ava", name=f"ava{j}")
                av_b = pav.tile([P, SQ], f32, tag="avb", name=f"avb{j}")
                pp0 = emit_scores(psc1, "sca", j, 0, 0)
                pp1 = emit_scores(psc1, "scb", j, 0, 1)
                for g in range(NG):
                    nxt = []
                    if g + 1 < NG:
                        nxt = [emit_scores(psc1, "sca", j, g + 1, 0),
                               emit_scores(psc1, "scb", j, g + 1, 1)]
                    emit_av(av_a, j, g, 0, pp0)
                    emit_av(av_b, j, g, 1, pp1)
                    if nxt:
                        pp0, pp1 = nxt
                pend[j] = emit_rowsum(j, av_a, av_b)
            for j in (6, 7):
                emit_attnT(j, pend.pop(j))
        pav.release()

        # ---------------- phase C: output projection ----------------
        # j=0..6 accumulate into 4 chunk psums first (these only need pairs
        # 0-6, so they fill the pair-7 rowsum latency), then j=7 + stores,
        # then the remaining 4 chunks. pop reuses the score-pool banks, which
        # free as soon as pair 7's last exp is read.
        with tc.tile_pool(name="pop", bufs=4, space="PSUM") as pop, \
             tc.tile_pool(name="osb", bufs=2) as osb:
            def c_chunk_mms(po, ch, js):
                st, nt2 = ch // 2, ch % 2
                for j in js:
                    nc.tensor.matmul(
                        po, lhsT=attnT_sb[:, j, st * P:(st + 1) * P],
                        rhs=wo_sb[:, j, nt2 * SQ:(nt2 + 1) * SQ],
                        start=(j == 0), stop=(j == 7))

            def c_chunk_out(po, ch):
                st, nt2 = ch // 2, ch % 2
                ob = osb.tile([P, SQ], f32, tag="ob", name=f"ob{ch}")
                nc.vector.tensor_copy(out=ob, in_=po)
                nc.sync.dma_start(out=out_r[st, nt2], in_=ob)

            pos = {}
            for ch in range(4):
                pos[ch] = pop.tile([P, SQ], f32, tag="po", name=f"po{ch}")
                c_chunk_mms(pos[ch], ch, range(7))
            for ch in range(4):
                c_chunk_mms(pos[ch], ch, [7])
                c_chunk_out(pos[ch], ch)
            for ch in range(4, 8):
                po = pop.tile([P, SQ], f32, tag="po", name=f"po{ch}")
                c_chunk_mms(po, ch, range(8))
                c_chunk_out(po, ch)
    persist.release()


def _build():
    if "nc" in _CACHE:
        return _CACHE["nc"]
    nc = bacc.Bacc("TRN2", target_bir_lowering=False, debug=False)
    xT = nc.dram_tensor("xT", [D, S], bf16, kind="ExternalInput").ap()
    wqT = nc.dram_tensor("wqT", [8, D, P], bf16, kind="ExternalInput").ap()
    wkT = nc.dram_tensor("wkT", [D, KVH * HD], bf16, kind="ExternalInput").ap()
    wvT = nc.dram_tensor("wvT", [D, KVH * HD], bf16, kind="ExternalInput").ap()
    woT = nc.dram_tensor("woT", [H * HD, D], bf16, kind="ExternalInput").ap()
    emT = nc.dram_tensor("emT", [2, S, SQ], bf16, kind="ExternalInput").ap()
    out = nc.dram_tensor("out", [SQ, D], f32, kind="ExternalOutput").ap()
    with tile.TileContext(nc) as tc:
        _body(tc, xT, wqT, wkT, wvT, woT, emT, out)
    nc.compile()
    _CACHE["nc"] = nc
    return nc


def _host_prep(hidden_states, full_mask, tag_mask, wq, wk, wv, wo):
    # pair-ordered feature permutation for wq columns / wo.T rows
    perm = np.concatenate([np.r_[a * HD:(a + 1) * HD, b * HD:(b + 1) * HD]
                           for a, b in PAIRS])
    import ml_dtypes
    bf = ml_dtypes.bfloat16
    wqTf = np.ascontiguousarray(wq.T[:, perm] * 0.125)             # [D, 1024]
    # [j, p, kc, f] layout so each per-pair chunk DMA reads 2KB/partition runs
    wqT = np.ascontiguousarray(
        wqTf.reshape(KC, P, 8, P).transpose(2, 1, 0, 3)).astype(bf)  # [8,128,8,128]
    wqT = np.ascontiguousarray(wqT.reshape(8, D, P))
    wkT = np.ascontiguousarray(wk.T).astype(bf)                    # [D, 256]
    wvT = np.ascontiguousarray(wv.T).astype(bf)                    # [D, 256]
    woT = np.ascontiguousarray(wo.T[perm, :]).astype(bf)           # [1024, D]
    # exp(mask) transposed to [sk, sq], rolled per core
    emasks = [np.exp(full_mask[b, 0].T) for b in range(B)] + \
             [np.exp(tag_mask[b, 0].T) for b in range(B)]
    xTs = [np.ascontiguousarray(hidden_states[b].T) for b in range(B)]
    in_maps = []
    for c in range(NCORES):
        b, q0 = c // 4, (c % 4) * SQ
        xT_c = np.roll(xTs[b], -q0, axis=1).astype(bf)
        fmT = np.roll(emasks[b][:, q0:q0 + SQ], -q0, axis=0)
        tgT = np.roll(emasks[2 + b][:, q0:q0 + SQ], -q0, axis=0)
        emT_c = np.ascontiguousarray(np.stack([fmT, tgT])).astype(bf)
        in_maps.append({"xT": np.ascontiguousarray(xT_c), "wqT": wqT, "wkT": wkT,
                        "wvT": wvT, "woT": woT, "emT": emT_c})
    return in_maps


def kernel(hidden_states, full_mask, tag_mask, wq, wk, wv, wo, _trace=False):
    args = [np.asarray(a, np.float32) for a in
            (hidden_states, full_mask, tag_mask, wq, wk, wv, wo)]
    nc = _build()
    in_maps = _host_prep(*args)
    try:
        res = run_bass_kernel_spmd(nc, in_maps, core_ids=list(range(NCORES)),
                                   trace=_trace)
    except ModuleNotFoundError:
        res = run_bass_kernel_spmd(nc, in_maps, core_ids=list(range(NCORES)))
    _CACHE["last_results"] = res
    full = np.empty((B, S, D), np.float32)
    for c in range(NCORES):
        b, q0 = c // 4, (c % 4) * SQ
        full[b, q0:q0 + SQ, :] = res.results[c]["out"]
    return full


# revision 37
# speedup vs baseline: 33996.6564x; 1.0414x over previous
"""Trainium2 Bass kernel for nn_JanusModel (sparse_attention, GQA, two mask groups).

Sharding: core c in [0,8) handles batch b=c//4 and query-row block q0=(c%4)*512.
Each core computes all 16 heads for its 512 query rows -> disjoint output slices,
no collectives. Heavy operands laid out on host (transposes/permutes, exp(mask),
1/sqrt(hd) folded into wq, bf16 casts).

On-device math per core (all bf16 operands, fp32 PSUM):
  qT/kT/v projections, scoresT = K @ qT (row-tiled head pairs, [sk, sq] layout),
  P = exp(scores) * expm (ACT exp + DVE bf16 mul), AV matmuls with a ones-column
  appended to V so row 64 of each AV psum accumulates the softmax denominator,
  reciprocal + DMA broadcast, output projection accumulated over head pairs.

Pipelining: pair 0's whole attention is interleaved into the projection loop so
the ACT engine starts exp work ~10us in; pairs 1-7 run a software-pipelined
PE emission (scores of group g+1 issued before AV of group g); attnT division
for pair j is deferred to pair j+2 so the rowsum DMA roundtrip never stalls PE.
"""

import os
import sys

import numpy as np

for _p in ("/opt/trn_rl_repo",):
    if os.path.isdir(_p) and _p not in sys.path:
        sys.path.insert(0, _p)

import concourse.bass as bass
import concourse.tile as tile
from concourse import bacc, mybir
from concourse.bass_utils import run_bass_kernel_spmd

B, S, D = 2, 2048, 1024
H, KVH, HD = 16, 4, 64
NCORES = 8
SQ = S // 4          # 512 query rows per core
P = 128
NKT = S // P         # 16 key tiles
KC = D // P          # 8 contraction chunks for projections
NG = 8               # 2-tile k groups per half

# Head pairs: (a, b) share a kT tile; a uses kv head 2*(j//4), b uses +1.
PAIRS = [(0, 4), (1, 5), (2, 6), (3, 7), (8, 12), (9, 13), (10, 14), (11, 15)]

f32 = mybir.dt.float32
bf16 = mybir.dt.bfloat16
EXP = mybir.ActivationFunctionType.Exp

_CACHE = {}


def _body(tc, xT, wqT, wkT, wvT, woT, emT, out):
    nc = tc.nc
    rs_dram = nc.dram_tensor("rs_scratch", [8, 2, SQ], f32).ap()
    xT_r = xT.rearrange("(kc p) (c s) -> c p kc s", p=P, s=SQ)   # [4,128,8,512]
    wqT_r = wqT.rearrange("j (p kc) f -> j p kc f", p=P)         # [8,128,8,128]
    wkT_r = wkT.rearrange("(kc p) f -> p kc f", p=P)             # [128,8,256]
    wvT_r = wvT.rearrange("(kc p) f -> p kc f", p=P)             # [128,8,256]
    woT_r = woT.rearrange("(j p) d -> p j d", p=P)               # [128,8,1024]
    emT_r = emT.rearrange("m (t p) q -> m p t q", p=P)           # [2,128,16,512]
    out_r = out.rearrange("(t p) (n q) -> t n p q", p=P, q=SQ)   # [4,2,128,512]

    persist = tc.alloc_tile_pool(name="persist", bufs=1)
    qT_sb = persist.tile([P, 8, SQ], bf16, name="qT_sb")      # pair j: a 0:64, b 64:128
    kT_sb = persist.tile([P, 2, S], bf16, name="kT_sb")       # jt: kv 2jt 0:64, 2jt+1 64:128
    v_sb = persist.tile([P, NKT, KVH, HD + 1], bf16, name="v_sb")  # col HD = ones
    em_sb = persist.tile([P, 2, NKT, SQ], bf16, name="em_sb")
    attnT_sb = persist.tile([P, 8, SQ], bf16, name="attnT_sb")
    wo_sb = persist.tile([P, 8, D], bf16, name="wo_sb")
    warm = persist.tile([1, 2], bf16, name="warm")

    # ---- all input DMAs, ordered for earliest compute start (one SP queue) ----
    pav = tc.alloc_tile_pool(name="pav", bufs=2, space="PSUM")
    with tc.tile_pool(name="xw", bufs=1) as xw, \
         tc.tile_pool(name="prp", bufs=3) as prp, \
         tc.tile_pool(name="ppp", bufs=3) as ppp, \
         tc.tile_pool(name="small", bufs=2) as small:
        x_sb = xw.tile([P, KC, S], bf16, name="x_sb")
        wq_sb = xw.tile([P, 8, KC, P], bf16, name="wq_sb")
        wk_sb = xw.tile([P, KC, KVH * HD], bf16, name="wk_sb")
        wv_sb = xw.tile([P, KC, KVH * HD], bf16, name="wv_sb")
        nc.sync.dma_start(out=wv_sb, in_=wvT_r)
        nc.sync.dma_start(out=x_sb[:, :, 0:P], in_=xT_r[0, :, :, 0:P])
        nc.sync.dma_start(out=x_sb[:, :, P:SQ], in_=xT_r[0, :, :, P:SQ])
        nc.sync.dma_start(out=wk_sb, in_=wkT_r)
        nc.sync.dma_start(out=wq_sb[:, 0], in_=wqT_r[0])
        nc.sync.dma_start(out=em_sb[:, 0, 0:8, :], in_=emT_r[0, :, 0:8, :])
        nc.sync.dma_start(out=x_sb[:, :, SQ:2 * SQ], in_=xT_r[1])
        nc.sync.dma_start(out=wq_sb[:, 1], in_=wqT_r[1])
        nc.sync.dma_start(out=em_sb[:, 0, 8:16, :], in_=emT_r[0, :, 8:16, :])
        nc.sync.dma_start(out=x_sb[:, :, 2 * SQ:3 * SQ], in_=xT_r[2])
        for j in (2, 3):
            nc.sync.dma_start(out=wq_sb[:, j], in_=wqT_r[j])
        nc.sync.dma_start(out=x_sb[:, :, 3 * SQ:4 * SQ], in_=xT_r[3])
        for j in (4, 5, 6, 7):
            nc.sync.dma_start(out=wq_sb[:, j], in_=wqT_r[j])
        nc.sync.dma_start(out=em_sb[:, 1], in_=emT_r[1])
        nc.sync.dma_start(out=wo_sb, in_=woT_r)

        # ACT exp table warm-up + ones column of v_sb
        nc.vector.memset(warm, 0.0)
        nc.scalar.activation(out=warm, in_=warm, func=EXP)
        nc.vector.memset(v_sb[:, :, :, HD:HD + 1], 1.0)

        # ---------- emission helpers ----------
        def emit_scores(pool, tag, j, t0, ntg, half):
            """ntg score matmuls for k-tiles t0.. -> exp -> mask mul; returns pp."""
            jt = j // 4
            r0, r1 = 64 * half, 64 * half + 64
            sc = pool.tile([P, ntg, SQ], f32, tag=tag, name=f"sc{j}_{t0}_{half}")
            for i in range(ntg):
                t = t0 + i
                nc.tensor.matmul(
                    sc[:, i, :], lhsT=kT_sb[r0:r1, jt, t * P:(t + 1) * P],
                    rhs=qT_sb[r0:r1, j, :], start=True, stop=True)
            pr = prp.tile([P, ntg, SQ], bf16, tag="pr", name=f"pr{j}_{t0}_{half}")
            nc.scalar.activation(out=pr, in_=sc, func=EXP)
            pp = ppp.tile([P, ntg, SQ], bf16, tag="pp", name=f"pp{j}_{t0}_{half}")
            nc.vector.tensor_mul(pp, pr, em_sb[:, j // 4, t0:t0 + ntg, :])
            return pp

        def emit_av(av, j, t0, ntg, half, pp):
            kv = 2 * (j // 4) + half
            for i in range(ntg):
                t = t0 + i
                nc.tensor.matmul(
                    av[0:HD + 1, :], lhsT=v_sb[:, t, kv, :], rhs=pp[:, i, :],
                    start=(t == 0), stop=(t == NKT - 1))

        def emit_rowsum(j, av_a, av_b):
            """reciprocal of row HD, DMA roundtrip broadcast; returns (bc, av_a, av_b).

            The roundtrip latency is hidden by deferring the attnT division
            to pair j+2 (av pool is double-buffered)."""
            rr = small.tile([P, 2, SQ], f32, tag="rr", name=f"rr{j}")
            nc.vector.reciprocal(out=rr[HD:HD + 1, 0, :], in_=av_a[HD:HD + 1, :])
            nc.vector.reciprocal(out=rr[HD:HD + 1, 1, :], in_=av_b[HD:HD + 1, :])
            nc.sync.dma_start(out=rs_dram[j], in_=rr[HD:HD + 1, :, :])
            bc = small.tile([P, SQ], f32, tag="bc", name=f"bc{j}")
            for half in range(2):
                row = rs_dram[j, half, :]
                bcast = bass.AP(tensor=row.tensor, offset=row.offset,
                                ap=[[0, 64]] + list(row.ap))
                nc.sync.dma_start(out=bc[64 * half:64 * half + 64, :], in_=bcast)
            return bc, av_a, av_b

        def emit_attnT(j, pend):
            bc, av_a, av_b = pend
            nc.vector.tensor_mul(attnT_sb[0:64, j, :], av_a[0:HD, :], bc[0:64, :])
            nc.vector.tensor_mul(attnT_sb[64:128, j, :], av_b[0:HD, :], bc[64:128, :])

        # ---------------- phase A + pairs 0,1 interleaved ----------------
        # Pairs 0 and 1 share one score psum buffer (exps ping-pong through
        # it), keeping ACT busy while the PE runs projection chains. AV
        # consumption lags scores until the v tiles exist; leftover pp tiles
        # are retained in the ppp pool.
        pend = {}
        pp_store = {}
        sc_next = {0: 0, 1: 0}   # next k-TILE (1-tile groups during phase A)
        av_next = {0: 0, 1: 0}
        with tc.tile_pool(name="pps", bufs=2, space="PSUM") as pps, \
             tc.tile_pool(name="psc0", bufs=2, space="PSUM") as psc0:
            avs = {j: (pav.tile([P, SQ], f32, tag="ava", name=f"ava{j}"),
                       pav.tile([P, SQ], f32, tag="avb", name=f"avb{j}"))
                   for j in (0, 1)}

            def pump(s, budget):
                # consume: AV for tiles whose v projection exists (t <= 2s+1)
                for j in (0, 1):
                    while av_next[j] < sc_next[j] and av_next[j] <= 2 * s + 1:
                        t = av_next[j]
                        for half in (0, 1):
                            pp = pp_store.pop((j, t, half))
                            emit_av(avs[j][half], j, t, 1, half, pp)
                        av_next[j] += 1
                # produce: scores for tiles whose kT chunk exists (t <= 4s+3)
                n = 0
                while n < budget:
                    cands = [j for j in (0, 1)
                             if sc_next[j] < NKT and sc_next[j] <= 4 * s + 3
                             and s >= j]
                    if not cands:
                        break
                    j = min(cands, key=lambda jj: sc_next[jj])
                    t = sc_next[j]
                    for half in (0, 1):
                        pp_store[(j, t, half)] = emit_scores(psc0, "sc", j, t, 1, half)
                    sc_next[j] += 1
                    n += 1

            BUDGET = [4, 5, 5, 4, 4, 4, 3, 3]
            for s in range(8):
                jt, ns = s // 4, s % 4
                # v projection tiles 2s, 2s+1 (natural [sk, feat] layout)
                for t in (2 * s, 2 * s + 1):
                    ps = pps.tile([P, KVH * HD], f32, tag="pa", name=f"psv{t}")
                    for kc in range(KC):
                        nc.tensor.matmul(
                            ps, lhsT=x_sb[:, kc, t * P:(t + 1) * P],
                            rhs=wv_sb[:, kc, :],
                            start=(kc == 0), stop=(kc == KC - 1))
                    nc.vector.tensor_copy(
                        out=v_sb[:, t, :, 0:HD],
                        in_=ps.rearrange("p (k h) -> p k h", h=HD))
                # k projection chunk: kT[:, jt, ns*SQ:(ns+1)*SQ]
                ps = pps.tile([P, SQ], f32, tag="pa", name=f"psk{s}")
                for kc in range(KC):
                    nc.tensor.matmul(
                        ps, lhsT=wk_sb[:, kc, jt * P:(jt + 1) * P],
                        rhs=x_sb[:, kc, ns * SQ:(ns + 1) * SQ],
                        start=(kc == 0), stop=(kc == KC - 1))
                nc.vector.tensor_copy(out=kT_sb[:, jt, ns * SQ:(ns + 1) * SQ], in_=ps)
                # q projection for pair s (scale folded into wq on host)
                ps = pps.tile([P, SQ], f32, tag="pa", name=f"psq{s}")
                for kc in range(KC):
                    nc.tensor.matmul(
                        ps, lhsT=wq_sb[:, s, kc, :], rhs=x_sb[:, kc, 0:SQ],
                        start=(kc == 0), stop=(kc == KC - 1))
                nc.vector.tensor_copy(out=qT_sb[:, s, :], in_=ps)
                pump(s, BUDGET[s])
            pump(8, 0)  # drain remaining AV work for pairs 0,1
            assert not pp_store and av_next == {0: NKT, 1: NKT}
            pend[0] = emit_rowsum(0, *avs[0])
            pend[1] = emit_rowsum(1, *avs[1])

        # ---------------- phase B: pairs 2-7, software pipelined ----------------
        with tc.tile_pool(name="psc1", bufs=1, space="PSUM") as psc1:
            for j in range(2, 8):
                if j - 2 in pend:
                    emit_attnT(j - 2, pend.pop(j - 2))
                av_a = pav.tile([P, SQ], f32, tag="ava", name=f"ava{j}")
                av_b = pav.tile([P, SQ], f32, tag="avb", name=f"avb{j}")
                pp0 = emit_scores(psc1, "sca", j, 0, 2, 0)
                pp1 = emit_scores(psc1, "scb", j, 0, 2, 1)
                for g in range(NG):
                    nxt = []
                    if g + 1 < NG:
                        nxt = [emit_scores(psc1, "sca", j, 2 * (g + 1), 2, 0),
                               emit_scores(psc1, "scb", j, 2 * (g + 1), 2, 1)]
                    emit_av(av_a, j, 2 * g, 2, 0, pp0)
                    emit_av(av_b, j, 2 * g, 2, 1, pp1)
                    if nxt:
                        pp0, pp1 = nxt
                pend[j] = emit_rowsum(j, av_a, av_b)
            for j in (6, 7):
                emit_attnT(j, pend.pop(j))
        pav.release()

        # ---------------- phase C: output projection ----------------
        # j=0..6 accumulate into 4 chunk psums first (these only need pairs
        # 0-6, so they fill the pair-7 rowsum latency), then j=7 + stores,
        # then the remaining 4 chunks. pop reuses the score-pool banks, which
        # free as soon as pair 7's last exp is read.
        with tc.tile_pool(name="pop", bufs=4, space="PSUM") as pop, \
             tc.tile_pool(name="osb", bufs=8) as osb:
            def c_chunk_mms(po, ch, js):
                st, nt2 = ch // 2, ch % 2
                for j in js:
                    nc.tensor.matmul(
                        po, lhsT=attnT_sb[:, j, st * P:(st + 1) * P],
                        rhs=wo_sb[:, j, nt2 * SQ:(nt2 + 1) * SQ],
                        start=(j == 0), stop=(j == 7))

            def c_chunk_out(po, ch):
                st, nt2 = ch // 2, ch % 2
                ob = osb.tile([P, SQ], f32, tag="ob", name=f"ob{ch}")
                nc.vector.tensor_copy(out=ob, in_=po)
                nc.sync.dma_start(out=out_r[st, nt2], in_=ob)

            pos = {}
            for ch in range(4):
                pos[ch] = pop.tile([P, SQ], f32, tag="po", name=f"po{ch}")
                c_chunk_mms(pos[ch], ch, range(7))
            for ch in range(4):
                c_chunk_mms(pos[ch], ch, [7])
                c_chunk_out(pos[ch], ch)
            for ch in range(4, 8):
                po = pop.tile([P, SQ], f32, tag="po", name=f"po{ch}")
                c_chunk_mms(po, ch, range(8))
                c_chunk_out(po, ch)
    persist.release()


def _build():
    if "nc" in _CACHE:
        return _CACHE["nc"]
    nc = bacc.Bacc("TRN2", target_bir_lowering=False, debug=False)
    xT = nc.dram_tensor("xT", [D, S], bf16, kind="ExternalInput").ap()
    wqT = nc.dram_tensor("wqT", [8, D, P], bf16, kind="ExternalInput").ap()
    wkT = nc.dram_tensor("wkT", [D, KVH * HD], bf16, kind="ExternalInput").ap()
    wvT = nc.dram_tensor("wvT", [D, KVH * HD], bf16, kind="ExternalInput").ap()
    woT = nc.dram_tensor("woT", [H * HD, D], bf16, kind="ExternalInput").ap()
    emT = nc.dram_tensor("emT", [2, S, SQ], bf16, kind="ExternalInput").ap()
    out = nc.dram_tensor("out", [SQ, D], f32, kind="ExternalOutput").ap()
    with tile.TileContext(nc) as tc:
        _body(tc, xT, wqT, wkT, wvT, woT, emT, out)
    nc.compile()
    _CACHE["nc"] = nc
    return nc


def _host_prep(hidden_states, full_mask, tag_mask, wq, wk, wv, wo):
    # pair-ordered feature permutation for wq columns / wo.T rows
    perm = np.concatenate([np.r_[a * HD:(a + 1) * HD, b * HD:(b + 1) * HD]
                           for a, b in PAIRS])
    import ml_dtypes
    bf = ml_dtypes.bfloat16
    wqTf = np.ascontiguousarray(wq.T[:, perm] * 0.125)             # [D, 1024]
    # [j, p, kc, f] layout so each per-pair chunk DMA reads 2KB/partition runs
    wqT = np.ascontiguousarray(
        wqTf.reshape(KC, P, 8, P).transpose(2, 1, 0, 3)).astype(bf)  # [8,128,8,128]
    wqT = np.ascontiguousarray(wqT.reshape(8, D, P))
    wkT = np.ascontiguousarray(wk.T).astype(bf)                    # [D, 256]
    wvT = np.ascontiguousarray(wv.T).astype(bf)                    # [D, 256]
    woT = np.ascontiguousarray(wo.T[perm, :]).astype(bf)           # [1024, D]
    # exp(mask) transposed to [sk, sq], rolled per core
    emasks = [np.exp(full_mask[b, 0].T) for b in range(B)] + \
             [np.exp(tag_mask[b, 0].T) for b in range(B)]
    xTs = [np.ascontiguousarray(hidden_states[b].T) for b in range(B)]
    in_maps = []
    for c in range(NCORES):
        b, q0 = c // 4, (c % 4) * SQ
        xT_c = np.roll(xTs[b], -q0, axis=1).astype(bf)
        fmT = np.roll(emasks[b][:, q0:q0 + SQ], -q0, axis=0)
        tgT = np.roll(emasks[2 + b][:, q0:q0 + SQ], -q0, axis=0)
        emT_c = np.ascontiguousarray(np.stack([fmT, tgT])).astype(bf)
        in_maps.append({"xT": np.ascontiguousarray(xT_c), "wqT": wqT, "wkT": wkT,
                        "wvT": wvT, "woT": woT, "emT": emT_c})
    return in_maps


def kernel(hidden_states, full_mask, tag_mask, wq, wk, wv, wo, _trace=False):
    args = [np.asarray(a, np.float32) for a in
            (hidden_states, full_mask, tag_mask, wq, wk, wv, wo)]
    nc = _build()
    in_maps = _host_prep(*args)
    try:
        res = run_bass_kernel_spmd(nc, in_maps, core_ids=list(range(NCORES)),
                                   trace=_trace)
    except ModuleNotFoundError:
        res = run_bass_kernel_spmd(nc, in_maps, core_ids=list(range(NCORES)))
    _CACHE["last_results"] = res
    full = np.empty((B, S, D), np.float32)
    for c in range(NCORES):
        b, q0 = c // 4, (c % 4) * SQ
        full[b, q0:q0 + SQ, :] = res.results[c]["out"]
    return full


# revision 51
# speedup vs baseline: 35083.4122x; 1.0320x over previous
"""Trainium2 Bass kernel for nn_JanusModel (sparse_attention, GQA, two mask groups).

Sharding: core c in [0,8) handles batch b=c//4 and query-row block q0=(c%4)*512.
Each core computes all 16 heads for its 512 query rows -> disjoint output slices,
no collectives. Heavy operands laid out on host (transposes/permutes, exp(mask),
1/sqrt(hd) folded into wq, bf16 casts).

On-device math per core (all bf16 operands, fp32 PSUM):
  qT/kT/v projections, scoresT = K @ qT (row-tiled head pairs, [sk, sq] layout),
  P = exp(scores) * expm (ACT exp + DVE bf16 mul), AV matmuls with a ones-column
  appended to V so row 64 of each AV psum accumulates the softmax denominator,
  reciprocal + DMA broadcast, output projection accumulated over head pairs.

Pipelining: pair 0's whole attention is interleaved into the projection loop so
the ACT engine starts exp work ~10us in; pairs 1-7 run a software-pipelined
PE emission (scores of group g+1 issued before AV of group g); attnT division
for pair j is deferred to pair j+2 so the rowsum DMA roundtrip never stalls PE.
"""

import os
import sys

import numpy as np

for _p in ("/opt/trn_rl_repo",):
    if os.path.isdir(_p) and _p not in sys.path:
        sys.path.insert(0, _p)

import concourse.bass as bass
import concourse.tile as tile
from concourse import bacc, mybir
from concourse.bass_utils import run_bass_kernel_spmd

B, S, D = 2, 2048, 1024
H, KVH, HD = 16, 4, 64
NCORES = 8
SQ = S // 4          # 512 query rows per core
P = 128
NKT = S // P         # 16 key tiles
KC = D // P          # 8 contraction chunks for projections
NG = 8               # 2-tile k groups per half

# Head pairs: (a, b) share a kT tile; a uses kv head 2*(j//4), b uses +1.
PAIRS = [(0, 4), (1, 5), (2, 6), (3, 7), (8, 12), (9, 13), (10, 14), (11, 15)]

f32 = mybir.dt.float32
bf16 = mybir.dt.bfloat16
EXP = mybir.ActivationFunctionType.Exp

_CACHE = {}


def _body(tc, xT, wqT, wkT, wvT, woT, emT, out):
    nc = tc.nc
    rs_dram = nc.dram_tensor("rs_scratch", [8, 2, SQ], bf16).ap()
    xT_r = xT.rearrange("(kc p) (c s) -> c p kc s", p=P, s=SQ)   # [4,128,8,512]
    wqT_r = wqT.rearrange("j (p kc) f -> j p kc f", p=P)         # [8,128,8,128]
    wkT_r = wkT.rearrange("(kc p) f -> p kc f", p=P)             # [128,8,256]
    wvT_r = wvT.rearrange("(kc p) f -> p kc f", p=P)             # [128,8,256]
    woT_r = woT.rearrange("(j p) d -> p j d", p=P)               # [128,8,1024]
    emT_r = emT.rearrange("m (t p) q -> m p t q", p=P)           # [2,128,16,512]
    out_r = out.rearrange("(t p) (n q) -> t n p q", p=P, q=SQ)   # [4,2,128,512]

    persist = tc.alloc_tile_pool(name="persist", bufs=1)
    qT_sb = persist.tile([P, 8, SQ], bf16, name="qT_sb")      # pair j: a 0:64, b 64:128
    kT_sb = persist.tile([P, 2, S], bf16, name="kT_sb")       # jt: kv 2jt 0:64, 2jt+1 64:128
    v_sb = persist.tile([P, NKT, KVH, HD + 1], bf16, name="v_sb")  # col HD = ones
    em_sb = persist.tile([P, 2, NKT, SQ], bf16, name="em_sb")
    attnT_sb = persist.tile([P, 8, SQ], bf16, name="attnT_sb")
    wo_sb = persist.tile([P, 8, D], bf16, name="wo_sb")
    warm = persist.tile([1, 2], bf16, name="warm")

    # ---- all input DMAs, ordered for earliest compute start (one SP queue) ----
    pav = tc.alloc_tile_pool(name="pav", bufs=1, space="PSUM")
    with tc.tile_pool(name="xw", bufs=1) as xw, \
         tc.tile_pool(name="prp", bufs=3) as prp, \
         tc.tile_pool(name="ppp", bufs=3) as ppp, \
         tc.tile_pool(name="avsb", bufs=3) as avsbp, \
         tc.tile_pool(name="small", bufs=2) as small:
        x_sb = xw.tile([P, KC, S], bf16, name="x_sb")
        wq_sb = xw.tile([P, 8, KC, P], bf16, name="wq_sb")
        wk_sb = xw.tile([P, KC, KVH * HD], bf16, name="wk_sb")
        wv_sb = xw.tile([P, KC, KVH * HD], bf16, name="wv_sb")
        nc.sync.dma_start(out=wv_sb, in_=wvT_r)
        nc.sync.dma_start(out=x_sb[:, :, 0:P], in_=xT_r[0, :, :, 0:P])
        nc.sync.dma_start(out=x_sb[:, :, P:SQ], in_=xT_r[0, :, :, P:SQ])
        nc.sync.dma_start(out=wk_sb, in_=wkT_r)
        nc.sync.dma_start(out=wq_sb[:, 0], in_=wqT_r[0])
        nc.sync.dma_start(out=em_sb[:, 0, 0:8, :], in_=emT_r[0, :, 0:8, :])
        nc.sync.dma_start(out=x_sb[:, :, SQ:2 * SQ], in_=xT_r[1])
        nc.sync.dma_start(out=wq_sb[:, 1], in_=wqT_r[1])
        nc.sync.dma_start(out=em_sb[:, 0, 8:16, :], in_=emT_r[0, :, 8:16, :])
        nc.sync.dma_start(out=x_sb[:, :, 2 * SQ:3 * SQ], in_=xT_r[2])
        for j in (2, 3):
            nc.sync.dma_start(out=wq_sb[:, j], in_=wqT_r[j])
        nc.sync.dma_start(out=x_sb[:, :, 3 * SQ:4 * SQ], in_=xT_r[3])
        for j in (4, 5, 6, 7):
            nc.sync.dma_start(out=wq_sb[:, j], in_=wqT_r[j])
        nc.sync.dma_start(out=em_sb[:, 1], in_=emT_r[1])
        nc.sync.dma_start(out=wo_sb, in_=woT_r)

        # ACT exp table warm-up + ones column of v_sb
        nc.vector.memset(warm, 0.0)
        nc.scalar.activation(out=warm, in_=warm, func=EXP)
        nc.vector.memset(v_sb[:, :, :, HD:HD + 1], 1.0)

        # ---------- emission helpers ----------
        def emit_scores(pool, tag, j, t0, ntg, half):
            """ntg score matmuls for k-tiles t0.. -> exp -> mask mul; returns pp."""
            jt = j // 4
            r0, r1 = 64 * half, 64 * half + 64
            sc = pool.tile([P, ntg, SQ], f32, tag=tag, name=f"sc{j}_{t0}_{half}")
            for i in range(ntg):
                t = t0 + i
                nc.tensor.matmul(
                    sc[:, i, :], lhsT=kT_sb[r0:r1, jt, t * P:(t + 1) * P],
                    rhs=qT_sb[r0:r1, j, :], start=True, stop=True)
            pr = prp.tile([P, ntg, SQ], bf16, tag="pr", name=f"pr{j}_{t0}_{half}")
            nc.scalar.activation(out=pr, in_=sc, func=EXP)
            pp = ppp.tile([P, ntg, SQ], bf16, tag="pp", name=f"pp{j}_{t0}_{half}")
            nc.vector.tensor_mul(pp, pr, em_sb[:, j // 4, t0:t0 + ntg, :])
            return pp

        def emit_av(av, j, t0, ntg, half, pp):
            kv = 2 * (j // 4) + half
            for i in range(ntg):
                t = t0 + i
                nc.tensor.matmul(
                    av[0:HD + 1, :], lhsT=v_sb[:, t, kv, :], rhs=pp[:, i, :],
                    start=(t == 0), stop=(t == NKT - 1))

        def emit_rowsum(j, av_a, av_b):
            """Copy AV numerators+rowsums psum->sbuf bf16 (the only readers of
            the av psum banks, so they free ~1.3us after the pair ends), then
            reciprocal from the copy, DMA roundtrip broadcast in bf16. The
            attnT division is deferred to pair j+2. Returns (bc, avsb)."""
            avsb = avsbp.tile([P, 2, SQ], bf16, tag="av", name=f"avsb{j}")
            nc.vector.tensor_copy(out=avsb[0:HD + 1, 0, :], in_=av_a[0:HD + 1, :])
            nc.vector.tensor_copy(out=avsb[0:HD + 1, 1, :], in_=av_b[0:HD + 1, :])
            rr = small.tile([P, 2, SQ], bf16, tag="rr", name=f"rr{j}")
            with nc.allow_low_precision(reason="bf16 softmax denominators"):
                nc.vector.reciprocal(out=rr[HD:HD + 1, 0, :],
                                     in_=avsb[HD:HD + 1, 0, :])
                nc.vector.reciprocal(out=rr[HD:HD + 1, 1, :],
                                     in_=avsb[HD:HD + 1, 1, :])
            nc.sync.dma_start(out=rs_dram[j], in_=rr[HD:HD + 1, :, :])
            bc = small.tile([P, SQ], bf16, tag="bc", name=f"bc{j}")
            for half in range(2):
                row = rs_dram[j, half, :]
                bcast = bass.AP(tensor=row.tensor, offset=row.offset,
                                ap=[[0, 64]] + list(row.ap))
                nc.sync.dma_start(out=bc[64 * half:64 * half + 64, :], in_=bcast)
            return bc, avsb

        def emit_attnT(j, pend):
            bc, avsb = pend
            nc.vector.tensor_mul(attnT_sb[0:64, j, :], avsb[0:HD, 0, :],
                                 bc[0:64, :])
            nc.vector.tensor_mul(attnT_sb[64:128, j, :], avsb[0:HD, 1, :],
                                 bc[64:128, :])

        # ---------------- phase A + pairs 0,1 interleaved ----------------
        # Pairs 0 and 1 share one score psum buffer (exps ping-pong through
        # it), keeping ACT busy while the PE runs projection chains. AV
        # consumption lags scores until the v tiles exist; leftover pp tiles
        # are retained in the ppp pool.
        pend = {}
        pp_store = {}
        sc_next = {0: 0, 1: 0}   # next k-TILE (1-tile groups during phase A)
        av_next = {0: 0, 1: 0}
        with tc.tile_pool(name="pps", bufs=2, space="PSUM") as pps, \
             tc.tile_pool(name="psc0", bufs=2, space="PSUM") as psc0, \
             tc.tile_pool(name="pav1", bufs=1, space="PSUM") as pav1:
            avs = {0: (pav.tile([P, SQ], f32, tag="ava", name="ava0"),
                       pav.tile([P, SQ], f32, tag="avb", name="avb0")),
                   1: (pav1.tile([P, SQ], f32, tag="ava", name="ava1"),
                       pav1.tile([P, SQ], f32, tag="avb", name="avb1"))}

            def pump(s, budget):
                # consume: AV for tiles whose v projection exists (t <= 2s+1)
                for j in (0, 1):
                    while av_next[j] < sc_next[j] and av_next[j] <= 2 * s + 1:
                        t = av_next[j]
                        for half in (0, 1):
                            pp = pp_store.pop((j, t, half))
                            emit_av(avs[j][half], j, t, 1, half, pp)
                        av_next[j] += 1
                # produce: scores for tiles whose kT chunk exists (t <= 4s+3)
                n = 0
                while n < budget:
                    cands = [j for j in (0, 1)
                             if sc_next[j] < NKT and sc_next[j] <= 4 * s + 3
                             and s >= j]
                    if not cands:
                        break
                    j = min(cands, key=lambda jj: sc_next[jj])
                    t = sc_next[j]
                    for half in (0, 1):
                        pp_store[(j, t, half)] = emit_scores(psc0, "sc", j, t, 1, half)
                    sc_next[j] += 1
                    n += 1

            BUDGET = [4, 5, 5, 4, 4, 4, 3, 3]
            for s in range(8):
                jt, ns = s // 4, s % 4
                # v projection tiles 2s, 2s+1 (natural [sk, feat] layout)
                for t in (2 * s, 2 * s + 1):
                    ps = pps.tile([P, KVH * HD], f32, tag="pa", name=f"psv{t}")
                    for kc in range(KC):
                        nc.tensor.matmul(
                            ps, lhsT=x_sb[:, kc, t * P:(t + 1) * P],
                            rhs=wv_sb[:, kc, :],
                            start=(kc == 0), stop=(kc == KC - 1))
                    nc.vector.tensor_copy(
                        out=v_sb[:, t, :, 0:HD],
                        in_=ps.rearrange("p (k h) -> p k h", h=HD))
                # k projection chunk: kT[:, jt, ns*SQ:(ns+1)*SQ]
                ps = pps.tile([P, SQ], f32, tag="pa", name=f"psk{s}")
                for kc in range(KC):
                    nc.tensor.matmul(
                        ps, lhsT=wk_sb[:, kc, jt * P:(jt + 1) * P],
                        rhs=x_sb[:, kc, ns * SQ:(ns + 1) * SQ],
                        start=(kc == 0), stop=(kc == KC - 1))
                nc.vector.tensor_copy(out=kT_sb[:, jt, ns * SQ:(ns + 1) * SQ], in_=ps)
                # q projection for pair s (scale folded into wq on host)
                ps = pps.tile([P, SQ], f32, tag="pa", name=f"psq{s}")
                for kc in range(KC):
                    nc.tensor.matmul(
                        ps, lhsT=wq_sb[:, s, kc, :], rhs=x_sb[:, kc, 0:SQ],
                        start=(kc == 0), stop=(kc == KC - 1))
                nc.vector.tensor_copy(out=qT_sb[:, s, :], in_=ps)
                pump(s, BUDGET[s])
            pump(8, 0)  # drain remaining AV work for pairs 0,1
            assert not pp_store and av_next == {0: NKT, 1: NKT}
            pend[0] = emit_rowsum(0, *avs[0])
            pend[1] = emit_rowsum(1, *avs[1])

        # ---------------- phase B: pairs 2-7, software pipelined ----------------
        BGROUPS = [(0, 3), (3, 3), (6, 3), (9, 3), (12, 3), (15, 1)]
        with tc.tile_pool(name="psc1", bufs=1, space="PSUM") as psc1:
            for j in range(2, 8):
                if j - 2 in pend:
                    emit_attnT(j - 2, pend.pop(j - 2))
                av_a = pav.tile([P, SQ], f32, tag="ava", name=f"ava{j}")
                av_b = pav.tile([P, SQ], f32, tag="avb", name=f"avb{j}")
                t00, n00 = BGROUPS[0]
                pp0 = emit_scores(psc1, "sca", j, t00, n00, 0)
                pp1 = emit_scores(psc1, "scb", j, t00, n00, 1)
                for gi, (t0g, ng) in enumerate(BGROUPS):
                    nxt = []
                    if gi + 1 < len(BGROUPS):
                        tn, nn2 = BGROUPS[gi + 1]
                        nxt = [emit_scores(psc1, "sca", j, tn, nn2, 0),
                               emit_scores(psc1, "scb", j, tn, nn2, 1)]
                    emit_av(av_a, j, t0g, ng, 0, pp0)
                    emit_av(av_b, j, t0g, ng, 1, pp1)
                    if nxt:
                        pp0, pp1 = nxt
                pend[j] = emit_rowsum(j, av_a, av_b)
            for j in (6, 7):
                emit_attnT(j, pend.pop(j))
        pav.release()

        # ---------------- phase C: output projection ----------------
        # j=0..6 accumulate into 4 chunk psums first (these only need pairs
        # 0-6, so they fill the pair-7 rowsum latency), then j=7 + stores,
        # then the remaining 4 chunks. pop reuses the score-pool banks, which
        # free as soon as pair 7's last exp is read.
        with tc.tile_pool(name="pop", bufs=4, space="PSUM") as pop, \
             tc.tile_pool(name="osb", bufs=8) as osb:
            def c_chunk_mms(po, ch, js):
                st, nt2 = ch // 2, ch % 2
                for j in js:
                    nc.tensor.matmul(
                        po, lhsT=attnT_sb[:, j, st * P:(st + 1) * P],
                        rhs=wo_sb[:, j, nt2 * SQ:(nt2 + 1) * SQ],
                        start=(j == 0), stop=(j == 7))

            def c_chunk_out(po, ch):
                st, nt2 = ch // 2, ch % 2
                ob = osb.tile([P, SQ], f32, tag="ob", name=f"ob{ch}")
                nc.vector.tensor_copy(out=ob, in_=po)
                nc.sync.dma_start(out=out_r[st, nt2], in_=ob)

            pos = {}
            for ch in range(4):
                pos[ch] = pop.tile([P, SQ], f32, tag="po", name=f"po{ch}")
                c_chunk_mms(pos[ch], ch, range(7))
            for ch in range(4):
                c_chunk_mms(pos[ch], ch, [7])
                c_chunk_out(pos[ch], ch)
            for ch in range(4, 8):
                po = pop.tile([P, SQ], f32, tag="po", name=f"po{ch}")
                c_chunk_mms(po, ch, range(8))
                c_chunk_out(po, ch)
    persist.release()


def _build():
    if "nc" in _CACHE:
        return _CACHE["nc"]
    nc = bacc.Bacc("TRN2", target_bir_lowering=False, debug=False)
    xT = nc.dram_tensor("xT", [D, S], bf16, kind="ExternalInput").ap()
    wqT = nc.dram_tensor("wqT", [8, D, P], bf16, kind="ExternalInput").ap()
    wkT = nc.dram_tensor("wkT", [D, KVH * HD], bf16, kind="ExternalInput").ap()
    wvT = nc.dram_tensor("wvT", [D, KVH * HD], bf16, kind="ExternalInput").ap()
    woT = nc.dram_tensor("woT", [H * HD, D], bf16, kind="ExternalInput").ap()
    emT = nc.dram_tensor("emT", [2, S, SQ], bf16, kind="ExternalInput").ap()
    out = nc.dram_tensor("out", [SQ, D], f32, kind="ExternalOutput").ap()
    with tile.TileContext(nc) as tc:
        _body(tc, xT, wqT, wkT, wvT, woT, emT, out)
    nc.compile()
    _CACHE["nc"] = nc
    return nc


def _host_prep(hidden_states, full_mask, tag_mask, wq, wk, wv, wo):
    # pair-ordered feature permutation for wq columns / wo.T rows
    perm = np.concatenate([np.r_[a * HD:(a + 1) * HD, b * HD:(b + 1) * HD]
                           for a, b in PAIRS])
    import ml_dtypes
    bf = ml_dtypes.bfloat16
    wqTf = np.ascontiguousarray(wq.T[:, perm] * 0.125)             # [D, 1024]
    # [j, p, kc, f] layout so each per-pair chunk DMA reads 2KB/partition runs
    wqT = np.ascontiguousarray(
        wqTf.reshape(KC, P, 8, P).transpose(2, 1, 0, 3)).astype(bf)  # [8,128,8,128]
    wqT = np.ascontiguousarray(wqT.reshape(8, D, P))
    wkT = np.ascontiguousarray(wk.T).astype(bf)                    # [D, 256]
    wvT = np.ascontiguousarray(wv.T).astype(bf)                    # [D, 256]
    woT = np.ascontiguousarray(wo.T[perm, :]).astype(bf)           # [1024, D]
    # exp(mask) transposed to [sk, sq], rolled per core
    emasks = [np.exp(full_mask[b, 0].T) for b in range(B)] + \
             [np.exp(tag_mask[b, 0].T) for b in range(B)]
    xTs = [np.ascontiguousarray(hidden_states[b].T) for b in range(B)]
    in_maps = []
    for c in range(NCORES):
        b, q0 = c // 4, (c % 4) * SQ
        xT_c = np.roll(xTs[b], -q0, axis=1).astype(bf)
        fmT = np.roll(emasks[b][:, q0:q0 + SQ], -q0, axis=0)
        tgT = np.roll(emasks[2 + b][:, q0:q0 + SQ], -q0, axis=0)
        emT_c = np.ascontiguousarray(np.stack([fmT, tgT])).astype(bf)
        in_maps.append({"xT": np.ascontiguousarray(xT_c), "wqT": wqT, "wkT": wkT,
                        "wvT": wvT, "woT": woT, "emT": emT_c})
    return in_maps


def kernel(hidden_states, full_mask, tag_mask, wq, wk, wv, wo, _trace=False):
    args = [np.asarray(a, np.float32) for a in
            (hidden_states, full_mask, tag_mask, wq, wk, wv, wo)]
    nc = _build()
    in_maps = _host_prep(*args)
    try:
        res = run_bass_kernel_spmd(nc, in_maps, core_ids=list(range(NCORES)),
                                   trace=_trace)
    except ModuleNotFoundError:
        res = run_bass_kernel_spmd(nc, in_maps, core_ids=list(range(NCORES)))
    _CACHE["last_results"] = res
    full = np.empty((B, S, D), np.float32)
    for c in range(NCORES):
        b, q0 = c // 4, (c % 4) * SQ
        full[b, q0:q0 + SQ, :] = res.results[c]["out"]
    return full


# revision 53
# speedup vs baseline: 36880.5823x; 1.0512x over previous
"""Trainium2 Bass kernel for nn_JanusModel (sparse_attention, GQA, two mask groups).

Sharding: core c in [0,8) handles batch b=c//4 and query-row block q0=(c%4)*512.
Each core computes all 16 heads for its 512 query rows -> disjoint output slices,
no collectives. Heavy operands laid out on host (transposes/permutes, exp(mask),
1/sqrt(hd) folded into wq, bf16 casts).

On-device math per core (all bf16 operands, fp32 PSUM):
  qT/kT/v projections, scoresT = K @ qT (row-tiled head pairs, [sk, sq] layout),
  P = exp(scores) * expm (ACT exp + DVE bf16 mul), AV matmuls with a ones-column
  appended to V so row 64 of each AV psum accumulates the softmax denominator,
  reciprocal + DMA broadcast, output projection accumulated over head pairs.

Pipelining: pair 0's whole attention is interleaved into the projection loop so
the ACT engine starts exp work ~10us in; pairs 1-7 run a software-pipelined
PE emission (scores of group g+1 issued before AV of group g); attnT division
for pair j is deferred to pair j+2 so the rowsum DMA roundtrip never stalls PE.
"""

import os
import sys

import numpy as np

for _p in ("/opt/trn_rl_repo",):
    if os.path.isdir(_p) and _p not in sys.path:
        sys.path.insert(0, _p)

import concourse.bass as bass
import concourse.tile as tile
from concourse import bacc, mybir
from concourse.bass_utils import run_bass_kernel_spmd

B, S, D = 2, 2048, 1024
H, KVH, HD = 16, 4, 64
NCORES = 8
SQ = S // 4          # 512 query rows per core
P = 128
NKT = S // P         # 16 key tiles
KC = D // P          # 8 contraction chunks for projections
NG = 8               # 2-tile k groups per half

# Head pairs: (a, b) share a kT tile; a uses kv head 2*(j//4), b uses +1.
PAIRS = [(0, 4), (1, 5), (2, 6), (3, 7), (8, 12), (9, 13), (10, 14), (11, 15)]

f32 = mybir.dt.float32
bf16 = mybir.dt.bfloat16
EXP = mybir.ActivationFunctionType.Exp

_CACHE = {}


def _body(tc, xT, wqT, wkT, wvT, woT, emT, out):
    nc = tc.nc
    rs_dram = nc.dram_tensor("rs_scratch", [8, 2, SQ], bf16).ap()
    xT_r = xT.rearrange("(kc p) (c s) -> c p kc s", p=P, s=SQ)   # [4,128,8,512]
    wqT_r = wqT.rearrange("j (p kc) f -> j p kc f", p=P)         # [8,128,8,128]
    wkT_r = wkT.rearrange("(kc p) f -> p kc f", p=P)             # [128,8,256]
    wvT_r = wvT.rearrange("(kc p) f -> p kc f", p=P)             # [128,8,256]
    woT_r = woT.rearrange("(j p) d -> p j d", p=P)               # [128,8,1024]
    emT_r = emT.rearrange("m (t p) q -> m p t q", p=P)           # [2,128,16,512]
    out_r = out.rearrange("(t p) (n q) -> t n p q", p=P, q=SQ)   # [4,2,128,512]

    persist = tc.alloc_tile_pool(name="persist", bufs=1)
    qT_sb = persist.tile([P, 8, SQ], bf16, name="qT_sb")      # pair j: a 0:64, b 64:128
    kT_sb = persist.tile([P, 2, S], bf16, name="kT_sb")       # jt: kv 2jt 0:64, 2jt+1 64:128
    v_sb = persist.tile([P, NKT, KVH, HD + 1], bf16, name="v_sb")  # col HD = ones
    em_sb = persist.tile([P, 2, NKT, SQ], bf16, name="em_sb")
    attnT_sb = persist.tile([P, 8, SQ], bf16, name="attnT_sb")
    wo_sb = persist.tile([P, 8, D], bf16, name="wo_sb")
    warm = persist.tile([1, 2], bf16, name="warm")

    # ---- all input DMAs, ordered for earliest compute start (one SP queue) ----
    pav = tc.alloc_tile_pool(name="pav", bufs=1, space="PSUM")
    with tc.tile_pool(name="xw", bufs=1) as xw, \
         tc.tile_pool(name="prp", bufs=3) as prp, \
         tc.tile_pool(name="ppp", bufs=3) as ppp, \
         tc.tile_pool(name="avsb", bufs=3) as avsbp, \
         tc.tile_pool(name="small", bufs=2) as small:
        x_sb = xw.tile([P, KC, S], bf16, name="x_sb")
        wq_sb = xw.tile([P, 8, KC, P], bf16, name="wq_sb")
        wk_sb = xw.tile([P, KC, KVH * HD], bf16, name="wk_sb")
        wv_sb = xw.tile([P, KC, KVH * HD], bf16, name="wv_sb")
        nc.sync.dma_start(out=wv_sb, in_=wvT_r)
        nc.sync.dma_start(out=x_sb[:, :, 0:P], in_=xT_r[0, :, :, 0:P])
        nc.sync.dma_start(out=x_sb[:, :, P:SQ], in_=xT_r[0, :, :, P:SQ])
        nc.sync.dma_start(out=wk_sb, in_=wkT_r)
        nc.sync.dma_start(out=wq_sb[:, 0], in_=wqT_r[0])
        nc.sync.dma_start(out=em_sb[:, 0, 0:8, :], in_=emT_r[0, :, 0:8, :])
        nc.sync.dma_start(out=x_sb[:, :, SQ:2 * SQ], in_=xT_r[1])
        nc.sync.dma_start(out=wq_sb[:, 1], in_=wqT_r[1])
        nc.sync.dma_start(out=em_sb[:, 0, 8:16, :], in_=emT_r[0, :, 8:16, :])
        nc.sync.dma_start(out=x_sb[:, :, 2 * SQ:3 * SQ], in_=xT_r[2])
        for j in (2, 3):
            nc.sync.dma_start(out=wq_sb[:, j], in_=wqT_r[j])
        nc.sync.dma_start(out=x_sb[:, :, 3 * SQ:4 * SQ], in_=xT_r[3])
        for j in (4, 5, 6, 7):
            nc.sync.dma_start(out=wq_sb[:, j], in_=wqT_r[j])
        nc.sync.dma_start(out=em_sb[:, 1], in_=emT_r[1])
        nc.sync.dma_start(out=wo_sb, in_=woT_r)

        # ACT exp table warm-up + ones column of v_sb
        nc.vector.memset(warm, 0.0)
        nc.scalar.activation(out=warm, in_=warm, func=EXP)
        nc.vector.memset(v_sb[:, :, :, HD:HD + 1], 1.0)

        # ---------- emission helpers ----------
        def emit_scores(pool, tag, j, t0, ntg, half):
            """ntg score matmuls for k-tiles t0.. -> exp -> mask mul; returns pp."""
            jt = j // 4
            r0, r1 = 64 * half, 64 * half + 64
            sc = pool.tile([P, ntg, SQ], f32, tag=tag, name=f"sc{j}_{t0}_{half}")
            for i in range(ntg):
                t = t0 + i
                nc.tensor.matmul(
                    sc[:, i, :], lhsT=kT_sb[r0:r1, jt, t * P:(t + 1) * P],
                    rhs=qT_sb[r0:r1, j, :], start=True, stop=True)
            pr = prp.tile([P, ntg, SQ], bf16, tag="pr", name=f"pr{j}_{t0}_{half}")
            nc.scalar.activation(out=pr, in_=sc, func=EXP)
            pp = ppp.tile([P, ntg, SQ], bf16, tag="pp", name=f"pp{j}_{t0}_{half}")
            nc.vector.tensor_mul(pp, pr, em_sb[:, j // 4, t0:t0 + ntg, :])
            return pp

        def emit_av(av, j, t0, ntg, half, pp):
            kv = 2 * (j // 4) + half
            for i in range(ntg):
                t = t0 + i
                nc.tensor.matmul(
                    av[0:HD + 1, :], lhsT=v_sb[:, t, kv, :], rhs=pp[:, i, :],
                    start=(t == 0), stop=(t == NKT - 1))

        def emit_rowsum(j, av_a, av_b):
            """Copy AV numerators+rowsums psum->sbuf bf16 (the only readers of
            the av psum banks, so they free ~1.3us after the pair ends), then
            reciprocal from the copy, DMA roundtrip broadcast in bf16. The
            attnT division is deferred to pair j+2. Returns (bc, avsb)."""
            avsb = avsbp.tile([P, 2, SQ], bf16, tag="av", name=f"avsb{j}")
            nc.vector.tensor_copy(out=avsb[0:HD + 1, 0, :], in_=av_a[0:HD + 1, :])
            nc.vector.tensor_copy(out=avsb[0:HD + 1, 1, :], in_=av_b[0:HD + 1, :])
            rr = small.tile([P, 2, SQ], bf16, tag="rr", name=f"rr{j}")
            with nc.allow_low_precision(reason="bf16 softmax denominators"):
                nc.vector.reciprocal(out=rr[HD:HD + 1, 0, :],
                                     in_=avsb[HD:HD + 1, 0, :])
                nc.vector.reciprocal(out=rr[HD:HD + 1, 1, :],
                                     in_=avsb[HD:HD + 1, 1, :])
            nc.sync.dma_start(out=rs_dram[j], in_=rr[HD:HD + 1, :, :])
            bc = small.tile([P, 2, SQ], bf16, tag="bc", name=f"bc{j}")
            for half in range(2):
                row = rs_dram[j, half, :]
                bcast = bass.AP(tensor=row.tensor, offset=row.offset,
                                ap=[[0, 64]] + list(row.ap))
                nc.sync.dma_start(out=bc[0:64, half, :], in_=bcast)
            return bc, avsb

        def emit_attnT(j, pend):
            bc, avsb = pend
            nc.vector.tensor_mul(attnT_sb[0:64, j, :], avsb[0:HD, 0, :],
                                 bc[0:64, 0, :])
            nc.vector.tensor_mul(attnT_sb[64:128, j, :], avsb[0:HD, 1, :],
                                 bc[0:64, 1, :])

        # ---------------- phase A + pairs 0,1 interleaved ----------------
        # Pairs 0 and 1 share one score psum buffer (exps ping-pong through
        # it), keeping ACT busy while the PE runs projection chains. AV
        # consumption lags scores until the v tiles exist; leftover pp tiles
        # are retained in the ppp pool.
        pend = {}
        pp_store = {}
        sc_next = {0: 0, 1: 0}   # next k-TILE (1-tile groups during phase A)
        av_next = {0: 0, 1: 0}
        with tc.tile_pool(name="pps", bufs=2, space="PSUM") as pps, \
             tc.tile_pool(name="psc0", bufs=2, space="PSUM") as psc0, \
             tc.tile_pool(name="pav1", bufs=1, space="PSUM") as pav1:
            avs = {0: (pav.tile([P, SQ], f32, tag="ava", name="ava0"),
                       pav.tile([P, SQ], f32, tag="avb", name="avb0")),
                   1: (pav1.tile([P, SQ], f32, tag="ava", name="ava1"),
                       pav1.tile([P, SQ], f32, tag="avb", name="avb1"))}

            def pump(s, budget):
                # consume: AV for tiles whose v projection exists (t <= 2s+1)
                for j in (0, 1):
                    while av_next[j] < sc_next[j] and av_next[j] <= 2 * s + 1:
                        t = av_next[j]
                        for half in (0, 1):
                            pp = pp_store.pop((j, t, half))
                            emit_av(avs[j][half], j, t, 1, half, pp)
                        av_next[j] += 1
                # produce: scores for tiles whose kT chunk exists (t <= 4s+3)
                n = 0
                while n < budget:
                    cands = [j for j in (0, 1)
                             if sc_next[j] < NKT and sc_next[j] <= 4 * s + 3
                             and s >= j]
                    if not cands:
                        break
                    j = min(cands, key=lambda jj: sc_next[jj])
                    t = sc_next[j]
                    for half in (0, 1):
                        pp_store[(j, t, half)] = emit_scores(psc0, "sc", j, t, 1, half)
                    sc_next[j] += 1
                    n += 1

            BUDGET = [4, 5, 5, 4, 4, 4, 3, 3]
            for s in range(8):
                jt, ns = s // 4, s % 4
                # v projection tiles 2s, 2s+1 (natural [sk, feat] layout)
                for t in (2 * s, 2 * s + 1):
                    ps = pps.tile([P, KVH * HD], f32, tag="pa", name=f"psv{t}")
                    for kc in range(KC):
                        nc.tensor.matmul(
                            ps, lhsT=x_sb[:, kc, t * P:(t + 1) * P],
                            rhs=wv_sb[:, kc, :],
                            start=(kc == 0), stop=(kc == KC - 1))
                    nc.vector.tensor_copy(
                        out=v_sb[:, t, :, 0:HD],
                        in_=ps.rearrange("p (k h) -> p k h", h=HD))
                # k projection chunk: kT[:, jt, ns*SQ:(ns+1)*SQ]
                ps = pps.tile([P, SQ], f32, tag="pa", name=f"psk{s}")
                for kc in range(KC):
                    nc.tensor.matmul(
                        ps, lhsT=wk_sb[:, kc, jt * P:(jt + 1) * P],
                        rhs=x_sb[:, kc, ns * SQ:(ns + 1) * SQ],
                        start=(kc == 0), stop=(kc == KC - 1))
                nc.vector.tensor_copy(out=kT_sb[:, jt, ns * SQ:(ns + 1) * SQ], in_=ps)
                # q projection for pair s (scale folded into wq on host)
                ps = pps.tile([P, SQ], f32, tag="pa", name=f"psq{s}")
                for kc in range(KC):
                    nc.tensor.matmul(
                        ps, lhsT=wq_sb[:, s, kc, :], rhs=x_sb[:, kc, 0:SQ],
                        start=(kc == 0), stop=(kc == KC - 1))
                nc.vector.tensor_copy(out=qT_sb[:, s, :], in_=ps)
                pump(s, BUDGET[s])
            pump(8, 0)  # drain remaining AV work for pairs 0,1
            assert not pp_store and av_next == {0: NKT, 1: NKT}
            pend[0] = emit_rowsum(0, *avs[0])
            pend[1] = emit_rowsum(1, *avs[1])

        # ---------------- phase B: pairs 2-7, software pipelined ----------------
        # Flat (pair, group) work list, scores emitted two items ahead so the
        # ACT exp pipeline never drains at pair boundaries.
        BGROUPS = [(0, 3), (3, 3), (6, 3), (9, 3), (12, 3), (15, 1)]
        seq = [(j, t0, ng) for j in range(2, 8) for (t0, ng) in BGROUPS]
        with tc.tile_pool(name="psc1", bufs=1, space="PSUM") as psc1:
            def b_scores(j, t0, ng):
                return (emit_scores(psc1, "sca", j, t0, ng, 0),
                        emit_scores(psc1, "scb", j, t0, ng, 1))

            avt = {}
            pps_q = {i: b_scores(*seq[i]) for i in range(2)}
            for i, (j, t0, ng) in enumerate(seq):
                if t0 == 0:
                    if j - 2 in pend:
                        emit_attnT(j - 2, pend.pop(j - 2))
                    avt[j] = (pav.tile([P, SQ], f32, tag="ava", name=f"ava{j}"),
                              pav.tile([P, SQ], f32, tag="avb", name=f"avb{j}"))
                if i + 2 < len(seq):
                    pps_q[i + 2] = b_scores(*seq[i + 2])
                pp0, pp1 = pps_q.pop(i)
                emit_av(avt[j][0], j, t0, ng, 0, pp0)
                emit_av(avt[j][1], j, t0, ng, 1, pp1)
                if t0 + ng == NKT:
                    pend[j] = emit_rowsum(j, *avt.pop(j))
            for j in (6, 7):
                emit_attnT(j, pend.pop(j))
        pav.release()

        # ---------------- phase C: output projection ----------------
        # j=0..6 accumulate into 4 chunk psums first (these only need pairs
        # 0-6, so they fill the pair-7 rowsum latency), then j=7 + stores,
        # then the remaining 4 chunks. pop reuses the score-pool banks, which
        # free as soon as pair 7's last exp is read.
        with tc.tile_pool(name="pop", bufs=4, space="PSUM") as pop, \
             tc.tile_pool(name="osb", bufs=8) as osb:
            def c_chunk_mms(po, ch, js):
                st, nt2 = ch // 2, ch % 2
                for j in js:
                    nc.tensor.matmul(
                        po, lhsT=attnT_sb[:, j, st * P:(st + 1) * P],
                        rhs=wo_sb[:, j, nt2 * SQ:(nt2 + 1) * SQ],
                        start=(j == 0), stop=(j == 7))

            def c_chunk_out(po, ch):
                st, nt2 = ch // 2, ch % 2
                ob = osb.tile([P, SQ], f32, tag="ob", name=f"ob{ch}")
                nc.vector.tensor_copy(out=ob, in_=po)
                nc.sync.dma_start(out=out_r[st, nt2], in_=ob)

            pos = {}
            for ch in range(4):
                pos[ch] = pop.tile([P, SQ], f32, tag="po", name=f"po{ch}")
                c_chunk_mms(pos[ch], ch, range(7))
            for ch in range(4):
                c_chunk_mms(pos[ch], ch, [7])
                c_chunk_out(pos[ch], ch)
            for ch in range(4, 8):
                po = pop.tile([P, SQ], f32, tag="po", name=f"po{ch}")
                c_chunk_mms(po, ch, range(8))
                c_chunk_out(po, ch)
    persist.release()


def _build():
    if "nc" in _CACHE:
        return _CACHE["nc"]
    nc = bacc.Bacc("TRN2", target_bir_lowering=False, debug=False)
    xT = nc.dram_tensor("xT", [D, S], bf16, kind="ExternalInput").ap()
    wqT = nc.dram_tensor("wqT", [8, D, P], bf16, kind="ExternalInput").ap()
    wkT = nc.dram_tensor("wkT", [D, KVH * HD], bf16, kind="ExternalInput").ap()
    wvT = nc.dram_tensor("wvT", [D, KVH * HD], bf16, kind="ExternalInput").ap()
    woT = nc.dram_tensor("woT", [H * HD, D], bf16, kind="ExternalInput").ap()
    emT = nc.dram_tensor("emT", [2, S, SQ], bf16, kind="ExternalInput").ap()
    out = nc.dram_tensor("out", [SQ, D], f32, kind="ExternalOutput").ap()
    with tile.TileContext(nc) as tc:
        _body(tc, xT, wqT, wkT, wvT, woT, emT, out)
    nc.compile()
    _CACHE["nc"] = nc
    return nc


def _host_prep(hidden_states, full_mask, tag_mask, wq, wk, wv, wo):
    # pair-ordered feature permutation for wq columns / wo.T rows
    perm = np.concatenate([np.r_[a * HD:(a + 1) * HD, b * HD:(b + 1) * HD]
                           for a, b in PAIRS])
    import ml_dtypes
    bf = ml_dtypes.bfloat16
    wqTf = np.ascontiguousarray(wq.T[:, perm] * 0.125)             # [D, 1024]
    # [j, p, kc, f] layout so each per-pair chunk DMA reads 2KB/partition runs
    wqT = np.ascontiguousarray(
        wqTf.reshape(KC, P, 8, P).transpose(2, 1, 0, 3)).astype(bf)  # [8,128,8,128]
    wqT = np.ascontiguousarray(wqT.reshape(8, D, P))
    wkT = np.ascontiguousarray(wk.T).astype(bf)                    # [D, 256]
    wvT = np.ascontiguousarray(wv.T).astype(bf)                    # [D, 256]
    woT = np.ascontiguousarray(wo.T[perm, :]).astype(bf)           # [1024, D]
    # exp(mask) transposed to [sk, sq], rolled per core
    emasks = [np.exp(full_mask[b, 0].T) for b in range(B)] + \
             [np.exp(tag_mask[b, 0].T) for b in range(B)]
    xTs = [np.ascontiguousarray(hidden_states[b].T) for b in range(B)]
    in_maps = []
    for c in range(NCORES):
        b, q0 = c // 4, (c % 4) * SQ
        xT_c = np.roll(xTs[b], -q0, axis=1).astype(bf)
        fmT = np.roll(emasks[b][:, q0:q0 + SQ], -q0, axis=0)
        tgT = np.roll(emasks[2 + b][:, q0:q0 + SQ], -q0, axis=0)
        emT_c = np.ascontiguousarray(np.stack([fmT, tgT])).astype(bf)
        in_maps.append({"xT": np.ascontiguousarray(xT_c), "wqT": wqT, "wkT": wkT,
                        "wvT": wvT, "woT": woT, "emT": emT_c})
    return in_maps


def kernel(hidden_states, full_mask, tag_mask, wq, wk, wv, wo, _trace=False):
    args = [np.asarray(a, np.float32) for a in
            (hidden_states, full_mask, tag_mask, wq, wk, wv, wo)]
    nc = _build()
    in_maps = _host_prep(*args)
    try:
        res = run_bass_kernel_spmd(nc, in_maps, core_ids=list(range(NCORES)),
                                   trace=_trace)
    except ModuleNotFoundError:
        res = run_bass_kernel_spmd(nc, in_maps, core_ids=list(range(NCORES)))
    _CACHE["last_results"] = res
    full = np.empty((B, S, D), np.float32)
    for c in range(NCORES):
        b, q0 = c // 4, (c % 4) * SQ
        full[b, q0:q0 + SQ, :] = res.results[c]["out"]
    return full


# revision 58
# speedup vs baseline: 36929.4798x; 1.0013x over previous
"""Trainium2 Bass kernel for nn_JanusModel (sparse_attention, GQA, two mask groups).

Sharding: core c in [0,8) handles batch b=c//4 and query-row block q0=(c%4)*512.
Each core computes all 16 heads for its 512 query rows -> disjoint output slices,
no collectives. Heavy operands laid out on host (transposes/permutes, exp(mask),
1/sqrt(hd) folded into wq, bf16 casts).

On-device math per core (all bf16 operands, fp32 PSUM):
  qT/kT/v projections, scoresT = K @ qT (row-tiled head pairs, [sk, sq] layout),
  P = exp(scores) * expm (ACT exp + DVE bf16 mul), AV matmuls with a ones-column
  appended to V so row 64 of each AV psum accumulates the softmax denominator,
  reciprocal + DMA broadcast, output projection accumulated over head pairs.

Pipelining: pair 0's whole attention is interleaved into the projection loop so
the ACT engine starts exp work ~10us in; pairs 1-7 run a software-pipelined
PE emission (scores of group g+1 issued before AV of group g); attnT division
for pair j is deferred to pair j+2 so the rowsum DMA roundtrip never stalls PE.
"""

import os
import sys

import numpy as np

for _p in ("/opt/trn_rl_repo",):
    if os.path.isdir(_p) and _p not in sys.path:
        sys.path.insert(0, _p)

import concourse.bass as bass
import concourse.tile as tile
from concourse import bacc, mybir
from concourse.bass_utils import run_bass_kernel_spmd

B, S, D = 2, 2048, 1024
H, KVH, HD = 16, 4, 64
NCORES = 8
SQ = S // 4          # 512 query rows per core
P = 128
NKT = S // P         # 16 key tiles
KC = D // P          # 8 contraction chunks for projections
NG = 8               # 2-tile k groups per half

# Head pairs: (a, b) share a kT tile; a uses kv head 2*(j//4), b uses +1.
PAIRS = [(0, 4), (1, 5), (2, 6), (3, 7), (8, 12), (9, 13), (10, 14), (11, 15)]

f32 = mybir.dt.float32
bf16 = mybir.dt.bfloat16
EXP = mybir.ActivationFunctionType.Exp

_CACHE = {}


def _body(tc, xT, wqT, wkT, wvT, woT, emT, out):
    nc = tc.nc
    rs_dram = nc.dram_tensor("rs_scratch", [8, 2, SQ], bf16).ap()
    xT_r = xT.rearrange("(kc p) (c s) -> c p kc s", p=P, s=SQ)   # [4,128,8,512]
    wqT_r = wqT.rearrange("j (p kc) f -> j p kc f", p=P)         # [8,128,8,128]
    wkT_r = wkT.rearrange("(kc p) f -> p kc f", p=P)             # [128,8,256]
    wvT_r = wvT.rearrange("(kc p) f -> p kc f", p=P)             # [128,8,256]
    woT_r = woT.rearrange("(j p) d -> p j d", p=P)               # [128,8,1024]
    emT_r = emT.rearrange("m (t p) q -> m p t q", p=P)           # [2,128,16,512]
    out_r = out.rearrange("(t p) (n q) -> t n p q", p=P, q=SQ)   # [4,2,128,512]

    persist = tc.alloc_tile_pool(name="persist", bufs=1)
    qT_sb = persist.tile([P, 8, SQ], bf16, name="qT_sb")      # pair j: a 0:64, b 64:128
    kT_sb = persist.tile([P, 2, S], bf16, name="kT_sb")       # jt: kv 2jt 0:64, 2jt+1 64:128
    v_sb = persist.tile([P, NKT, KVH, HD + 1], bf16, name="v_sb")  # col HD = ones
    em_sb = persist.tile([P, 2, NKT, SQ], bf16, name="em_sb")
    attnT_sb = persist.tile([P, 8, SQ], bf16, name="attnT_sb")
    wo_sb = persist.tile([P, 8, D], bf16, name="wo_sb")
    warm = persist.tile([1, 2], bf16, name="warm")

    # ---- all input DMAs, ordered for earliest compute start (one SP queue) ----
    pav = tc.alloc_tile_pool(name="pav", bufs=1, space="PSUM")
    with tc.tile_pool(name="xw", bufs=1) as xw, \
         tc.tile_pool(name="prp", bufs=3) as prp, \
         tc.tile_pool(name="ppp", bufs=6) as ppp, \
         tc.tile_pool(name="avsb", bufs=3) as avsbp, \
         tc.tile_pool(name="small", bufs=2) as small:
        x_sb = xw.tile([P, KC, S], bf16, name="x_sb")
        wq_sb = xw.tile([P, 8, KC, P], bf16, name="wq_sb")
        wk_sb = xw.tile([P, KC, KVH * HD], bf16, name="wk_sb")
        wv_sb = xw.tile([P, KC, KVH * HD], bf16, name="wv_sb")
        nc.sync.dma_start(out=wv_sb, in_=wvT_r)
        nc.sync.dma_start(out=x_sb[:, :, 0:P], in_=xT_r[0, :, :, 0:P])
        nc.sync.dma_start(out=x_sb[:, :, P:SQ], in_=xT_r[0, :, :, P:SQ])
        nc.sync.dma_start(out=wk_sb, in_=wkT_r)
        nc.sync.dma_start(out=wq_sb[:, 0], in_=wqT_r[0])
        nc.sync.dma_start(out=em_sb[:, 0, 0:8, :], in_=emT_r[0, :, 0:8, :])
        nc.sync.dma_start(out=x_sb[:, :, SQ:2 * SQ], in_=xT_r[1])
        nc.sync.dma_start(out=wq_sb[:, 1], in_=wqT_r[1])
        nc.sync.dma_start(out=em_sb[:, 0, 8:16, :], in_=emT_r[0, :, 8:16, :])
        nc.sync.dma_start(out=x_sb[:, :, 2 * SQ:3 * SQ], in_=xT_r[2])
        for j in (2, 3):
            nc.sync.dma_start(out=wq_sb[:, j], in_=wqT_r[j])
        nc.sync.dma_start(out=x_sb[:, :, 3 * SQ:4 * SQ], in_=xT_r[3])
        for j in (4, 5, 6, 7):
            nc.sync.dma_start(out=wq_sb[:, j], in_=wqT_r[j])
        nc.sync.dma_start(out=em_sb[:, 1], in_=emT_r[1])
        nc.sync.dma_start(out=wo_sb, in_=woT_r)

        # ACT exp table warm-up + ones column of v_sb
        nc.vector.memset(warm, 0.0)
        nc.scalar.activation(out=warm, in_=warm, func=EXP)
        nc.vector.memset(v_sb[:, :, :, HD:HD + 1], 1.0)

        # ---------- emission helpers ----------
        def emit_scores(pool, tag, j, t0, ntg, half):
            """ntg score matmuls for k-tiles t0.. -> exp -> mask mul; returns pp."""
            jt = j // 4
            r0, r1 = 64 * half, 64 * half + 64
            sc = pool.tile([P, ntg, SQ], f32, tag=tag, name=f"sc{j}_{t0}_{half}")
            for i in range(ntg):
                t = t0 + i
                nc.tensor.matmul(
                    sc[:, i, :], lhsT=kT_sb[r0:r1, jt, t * P:(t + 1) * P],
                    rhs=qT_sb[r0:r1, j, :], start=True, stop=True)
            pr = prp.tile([P, ntg, SQ], bf16, tag="pr", name=f"pr{j}_{t0}_{half}")
            nc.scalar.activation(out=pr, in_=sc, func=EXP)
            pp = ppp.tile([P, ntg, SQ], bf16, tag="pp", name=f"pp{j}_{t0}_{half}")
            nc.vector.tensor_mul(pp, pr, em_sb[:, j // 4, t0:t0 + ntg, :])
            return pp

        def emit_av(av, j, t0, ntg, half, pp):
            kv = 2 * (j // 4) + half
            for i in range(ntg):
                t = t0 + i
                nc.tensor.matmul(
                    av[0:HD + 1, :], lhsT=v_sb[:, t, kv, :], rhs=pp[:, i, :],
                    start=(t == 0), stop=(t == NKT - 1))

        def emit_rowsum(j, av_a, av_b):
            """Copy AV numerators+rowsums psum->sbuf bf16 (the only readers of
            the av psum banks, so they free ~1.3us after the pair ends), then
            reciprocal from the copy, DMA roundtrip broadcast in bf16. The
            attnT division is deferred to pair j+2. Returns (bc, avsb)."""
            avsb = avsbp.tile([P, 2, SQ], bf16, tag="av", name=f"avsb{j}")
            nc.vector.tensor_copy(out=avsb[0:HD + 1, 0, :], in_=av_a[0:HD + 1, :])
            nc.vector.tensor_copy(out=avsb[0:HD + 1, 1, :], in_=av_b[0:HD + 1, :])
            rr = small.tile([P, 2, SQ], bf16, tag="rr", name=f"rr{j}")
            with nc.allow_low_precision(reason="bf16 softmax denominators"):
                nc.vector.reciprocal(out=rr[HD:HD + 1, 0, :],
                                     in_=avsb[HD:HD + 1, 0, :])
                nc.vector.reciprocal(out=rr[HD:HD + 1, 1, :],
                                     in_=avsb[HD:HD + 1, 1, :])
            nc.sync.dma_start(out=rs_dram[j], in_=rr[HD:HD + 1, :, :])
            bc = small.tile([P, 2, SQ], bf16, tag="bc", name=f"bc{j}")
            for half in range(2):
                row = rs_dram[j, half, :]
                bcast = bass.AP(tensor=row.tensor, offset=row.offset,
                                ap=[[0, 64]] + list(row.ap))
                nc.sync.dma_start(out=bc[0:64, half, :], in_=bcast)
            return bc, avsb

        def emit_attnT(j, pend):
            bc, avsb = pend
            nc.vector.tensor_mul(attnT_sb[0:64, j, :], avsb[0:HD, 0, :],
                                 bc[0:64, 0, :])
            nc.vector.tensor_mul(attnT_sb[64:128, j, :], avsb[0:HD, 1, :],
                                 bc[0:64, 1, :])

        # ---------------- phase A + pairs 0,1 interleaved ----------------
        # Pairs 0 and 1 share one score psum buffer (exps ping-pong through
        # it), keeping ACT busy while the PE runs projection chains. AV
        # consumption lags scores until the v tiles exist; leftover pp tiles
        # are retained in the ppp pool.
        pend = {}
        pp_store = {}
        sc_next = {0: 0, 1: 0}   # next k-TILE (1-tile groups during phase A)
        av_next = {0: 0, 1: 0}
        with tc.tile_pool(name="pps", bufs=2, space="PSUM") as pps, \
             tc.tile_pool(name="psc0", bufs=2, space="PSUM") as psc0, \
             tc.tile_pool(name="pav1", bufs=1, space="PSUM") as pav1:
            avs = {0: (pav.tile([P, SQ], f32, tag="ava", name="ava0"),
                       pav.tile([P, SQ], f32, tag="avb", name="avb0")),
                   1: (pav1.tile([P, SQ], f32, tag="ava", name="ava1"),
                       pav1.tile([P, SQ], f32, tag="avb", name="avb1"))}

            def pump(s, budget):
                # consume: AV for tiles whose v projection exists (t <= 2s+1)
                for j in (0, 1):
                    while av_next[j] < sc_next[j] and av_next[j] <= 2 * s + 1:
                        t = av_next[j]
                        for half in (0, 1):
                            pp = pp_store.pop((j, t, half))
                            emit_av(avs[j][half], j, t, 1, half, pp)
                        av_next[j] += 1
                # produce: scores for tiles whose kT chunk exists (t <= 4s+3)
                n = 0
                while n < budget:
                    cands = [j for j in (0, 1)
                             if sc_next[j] < NKT and sc_next[j] <= 4 * s + 3
                             and s >= j]
                    if not cands:
                        break
                    j = min(cands, key=lambda jj: sc_next[jj])
                    t = sc_next[j]
                    for half in (0, 1):
                        pp_store[(j, t, half)] = emit_scores(psc0, "sc", j, t, 1, half)
                    sc_next[j] += 1
                    n += 1

            BUDGET = [4, 5, 5, 4, 4, 4, 3, 3]
            for s in range(8):
                jt, ns = s // 4, s % 4
                # v projection tiles 2s, 2s+1 (natural [sk, feat] layout)
                for t in (2 * s, 2 * s + 1):
                    ps = pps.tile([P, KVH * HD], f32, tag="pa", name=f"psv{t}")
                    for kc in range(KC):
                        nc.tensor.matmul(
                            ps, lhsT=x_sb[:, kc, t * P:(t + 1) * P],
                            rhs=wv_sb[:, kc, :],
                            start=(kc == 0), stop=(kc == KC - 1))
                    nc.vector.tensor_copy(
                        out=v_sb[:, t, :, 0:HD],
                        in_=ps.rearrange("p (k h) -> p k h", h=HD))
                # k projection chunk: kT[:, jt, ns*SQ:(ns+1)*SQ]
                ps = pps.tile([P, SQ], f32, tag="pa", name=f"psk{s}")
                for kc in range(KC):
                    nc.tensor.matmul(
                        ps, lhsT=wk_sb[:, kc, jt * P:(jt + 1) * P],
                        rhs=x_sb[:, kc, ns * SQ:(ns + 1) * SQ],
                        start=(kc == 0), stop=(kc == KC - 1))
                nc.vector.tensor_copy(out=kT_sb[:, jt, ns * SQ:(ns + 1) * SQ], in_=ps)
                # q projection for pair s (scale folded into wq on host)
                ps = pps.tile([P, SQ], f32, tag="pa", name=f"psq{s}")
                for kc in range(KC):
                    nc.tensor.matmul(
                        ps, lhsT=wq_sb[:, s, kc, :], rhs=x_sb[:, kc, 0:SQ],
                        start=(kc == 0), stop=(kc == KC - 1))
                nc.vector.tensor_copy(out=qT_sb[:, s, :], in_=ps)
                pump(s, BUDGET[s])
            pump(8, 0)  # drain remaining AV work for pairs 0,1
            assert not pp_store and av_next == {0: NKT, 1: NKT}
            # bridge: score pair 2's first tiles in the still-live psc0 pool
            # so ACT keeps flowing while psc1's banks wait on pool release
            for t in (0, 1):
                for half in (0, 1):
                    pp_store[(2, t, half)] = emit_scores(psc0, "sc", 2, t, 1, half)
            pend[0] = emit_rowsum(0, *avs[0])
            pend[1] = emit_rowsum(1, *avs[1])

        # ---------------- phase B: pairs 2-7, software pipelined ----------------
        # Flat (pair, group) work list, scores emitted two items ahead so the
        # ACT exp pipeline never drains at pair boundaries. Pair 2's tiles
        # 0-1 were already scored into psc0 at the end of phase A.
        BGROUPS = [(0, 3), (3, 3), (6, 3), (9, 3), (12, 3), (15, 1)]
        seq = [(2, t0, ng) for (t0, ng) in
               [(2, 1), (3, 3), (6, 3), (9, 3), (12, 3), (15, 1)]] + \
              [(j, t0, ng) for j in range(3, 8) for (t0, ng) in BGROUPS]
        with tc.tile_pool(name="psc1", bufs=1, space="PSUM") as psc1:
            def b_scores(j, t0, ng):
                return (emit_scores(psc1, "sca", j, t0, ng, 0),
                        emit_scores(psc1, "scb", j, t0, ng, 1))

            avt = {}
            prev_j = None
            pps_q = {i: b_scores(*seq[i]) for i in range(2)}
            for i, (j, t0, ng) in enumerate(seq):
                if j != prev_j:
                    prev_j = j
                    if j - 2 in pend:
                        emit_attnT(j - 2, pend.pop(j - 2))
                    avt[j] = (pav.tile([P, SQ], f32, tag="ava", name=f"ava{j}"),
                              pav.tile([P, SQ], f32, tag="avb", name=f"avb{j}"))
                    if j == 2:
                        for tt in (0, 1):
                            for half in (0, 1):
                                emit_av(avt[2][half], 2, tt, 1, half,
                                        pp_store.pop((2, tt, half)))
                if i + 2 < len(seq):
                    pps_q[i + 2] = b_scores(*seq[i + 2])
                pp0, pp1 = pps_q.pop(i)
                emit_av(avt[j][0], j, t0, ng, 0, pp0)
                emit_av(avt[j][1], j, t0, ng, 1, pp1)
                if t0 + ng == NKT:
                    pend[j] = emit_rowsum(j, *avt.pop(j))
            for j in (6, 7):
                emit_attnT(j, pend.pop(j))
        pav.release()

        # ---------------- phase C: output projection ----------------
        # j=0..6 accumulate into 4 chunk psums first (these only need pairs
        # 0-6, so they fill the pair-7 rowsum latency), then j=7 + stores,
        # then the remaining 4 chunks. pop reuses the score-pool banks, which
        # free as soon as pair 7's last exp is read.
        with tc.tile_pool(name="pop", bufs=4, space="PSUM") as pop, \
             tc.tile_pool(name="osb", bufs=8) as osb:
            def c_chunk_mms(po, ch, js):
                st, nt2 = ch // 2, ch % 2
                for j in js:
                    nc.tensor.matmul(
                        po, lhsT=attnT_sb[:, j, st * P:(st + 1) * P],
                        rhs=wo_sb[:, j, nt2 * SQ:(nt2 + 1) * SQ],
                        start=(j == 0), stop=(j == 7))

            def c_chunk_out(po, ch):
                st, nt2 = ch // 2, ch % 2
                ob = osb.tile([P, SQ], f32, tag="ob", name=f"ob{ch}")
                nc.vector.tensor_copy(out=ob, in_=po)
                nc.sync.dma_start(out=out_r[st, nt2], in_=ob)

            pos = {}
            for ch in range(4):
                pos[ch] = pop.tile([P, SQ], f32, tag="po", name=f"po{ch}")
                c_chunk_mms(pos[ch], ch, range(7))
            for ch in range(4):
                c_chunk_mms(pos[ch], ch, [7])
                c_chunk_out(pos[ch], ch)
            for ch in range(4, 8):
                po = pop.tile([P, SQ], f32, tag="po", name=f"po{ch}")
                c_chunk_mms(po, ch, range(8))
                c_chunk_out(po, ch)
    persist.release()


def _build():
    if "nc" in _CACHE:
        return _CACHE["nc"]
    nc = bacc.Bacc("TRN2", target_bir_lowering=False, debug=False)
    xT = nc.dram_tensor("xT", [D, S], bf16, kind="ExternalInput").ap()
    wqT = nc.dram_tensor("wqT", [8, D, P], bf16, kind="ExternalInput").ap()
    wkT = nc.dram_tensor("wkT", [D, KVH * HD], bf16, kind="ExternalInput").ap()
    wvT = nc.dram_tensor("wvT", [D, KVH * HD], bf16, kind="ExternalInput").ap()
    woT = nc.dram_tensor("woT", [H * HD, D], bf16, kind="ExternalInput").ap()
    emT = nc.dram_tensor("emT", [2, S, SQ], bf16, kind="ExternalInput").ap()
    out = nc.dram_tensor("out", [SQ, D], f32, kind="ExternalOutput").ap()
    with tile.TileContext(nc) as tc:
        _body(tc, xT, wqT, wkT, wvT, woT, emT, out)
    nc.compile()
    _CACHE["nc"] = nc
    return nc


def _host_prep(hidden_states, full_mask, tag_mask, wq, wk, wv, wo):
    # pair-ordered feature permutation for wq columns / wo.T rows
    perm = np.concatenate([np.r_[a * HD:(a + 1) * HD, b * HD:(b + 1) * HD]
                           for a, b in PAIRS])
    import ml_dtypes
    bf = ml_dtypes.bfloat16
    wqTf = np.ascontiguousarray(wq.T[:, perm] * 0.125)             # [D, 1024]
    # [j, p, kc, f] layout so each per-pair chunk DMA reads 2KB/partition runs
    wqT = np.ascontiguousarray(
        wqTf.reshape(KC, P, 8, P).transpose(2, 1, 0, 3)).astype(bf)  # [8,128,8,128]
    wqT = np.ascontiguousarray(wqT.reshape(8, D, P))
    wkT = np.ascontiguousarray(wk.T).astype(bf)                    # [D, 256]
    wvT = np.ascontiguousarray(wv.T).astype(bf)                    # [D, 256]
    woT = np.ascontiguousarray(wo.T[perm, :]).astype(bf)           # [1024, D]
    # exp(mask) transposed to [sk, sq], rolled per core
    emasks = [np.exp(full_mask[b, 0].T) for b in range(B)] + \
             [np.exp(tag_mask[b, 0].T) for b in range(B)]
    xTs = [np.ascontiguousarray(hidden_states[b].T) for b in range(B)]
    in_maps = []
    for c in range(NCORES):
        b, q0 = c // 4, (c % 4) * SQ
        xT_c = np.roll(xTs[b], -q0, axis=1).astype(bf)
        fmT = np.roll(emasks[b][:, q0:q0 + SQ], -q0, axis=0)
        tgT = np.roll(emasks[2 + b][:, q0:q0 + SQ], -q0, axis=0)
        emT_c = np.ascontiguousarray(np.stack([fmT, tgT])).astype(bf)
        in_maps.append({"xT": np.ascontiguousarray(xT_c), "wqT": wqT, "wkT": wkT,
                        "wvT": wvT, "woT": woT, "emT": emT_c})
    return in_maps


def kernel(hidden_states, full_mask, tag_mask, wq, wk, wv, wo, _trace=False):
    args = [np.asarray(a, np.float32) for a in
            (hidden_states, full_mask, tag_mask, wq, wk, wv, wo)]
    nc = _build()
    in_maps = _host_prep(*args)
    try:
        res = run_bass_kernel_spmd(nc, in_maps, core_ids=list(range(NCORES)),
                                   trace=_trace)
    except ModuleNotFoundError:
        res = run_bass_kernel_spmd(nc, in_maps, core_ids=list(range(NCORES)))
    _CACHE["last_results"] = res
    full = np.empty((B, S, D), np.float32)
    for c in range(NCORES):
        b, q0 = c // 4, (c % 4) * SQ
        full[b, q0:q0 + SQ, :] = res.results[c]["out"]
    return full
